# revision 1
# baseline (speedup 1.0000x reference)
"""Bidirectional-LSTM (degenerate variant) Trainium2 kernel.

Reference semantics (see harness): for the forward direction only the last
timestep matters (h/c never update), and the backward direction is an
h-only recurrence (c stays zero), so only the i/g/o gates are ever used:

    h_fwd = sig(o) * tanh(sig(i) * tanh(g)),  gates = x_last @ W_ih_f.T + b_f
    h_bwd: scan t = S-1..0 with
        gates = x_t @ W_ih_b.T + b_b + h @ W_hh_b.T   (f-gate unused)
        h     = sig(o) * tanh(sig(i) * tanh(g))
    out = [h_fwd | h_bwd]  -> [256, 4096]

Distribution: pure data-parallel over batch (32 per core, 8 cores), weights
replicated. Per core:
  pass A : embedding gather (indirect DMA) + PE-transpose of X -> XT in DRAM
  phase 1: input projection xg = X @ Wi + b in fp32r (full fp32 inputs),
           stored bf16; forward cell folded in
  phase R: 128-step recurrence. gates = Wr.T @ h via 4 col-tiled concurrent
           M=32 matmuls (bf16), + xg, activations, PE-transpose of h for the
           next step's stationary operand.

Gate columns are host-permuted into 4 groups of (i|g|o) x 512 hid dims so
each PSUM column-group j directly yields h[:, 512j:512j+512].
"""

import numpy as np
import ml_dtypes

import concourse.bass as bass
import concourse.bacc as bacc
import concourse.mybir as mybir
import concourse.tile as tile
from concourse.masks import make_identity

VOCAB, EMB, HID = 50000, 1024, 2048
BATCH, SEQ = 256, 128
NCORES = 8
BLOC = BATCH // NCORES            # 32 batch rows per core
NTOK = BLOC * SEQ                 # 4096 tokens per core
NG = 4                            # PSUM column groups
GC = 3 * HID // NG                # 1536 gate cols per group (i|g|o x 512)
HG = HID // NG                    # 512 hid dims per group
G3 = 3 * HID                      # 6144 total igo gate cols
MT = NTOK // 128                  # 32 token m-tiles
KT_E = EMB // 128                 # 8 k-tiles for input projection
KT_H = HID // 128                 # 16 k-tiles for recurrence

F32 = mybir.dt.float32
F32R = mybir.dt.float32r
BF16 = mybir.dt.bfloat16
I32 = mybir.dt.int32

N_STEPS = SEQ  # overridable for mini builds


def build(n_steps=None):
    n_steps = n_steps or N_STEPS
    nc = bacc.Bacc("TRN2", target_bir_lowering=False, debug=False,
                   num_devices=NCORES)

    tok = nc.dram_tensor("tok", [NTOK, 1], I32, kind="ExternalInput")
    table = nc.dram_tensor("table", [VOCAB, EMB], F32R, kind="ExternalInput")
    Wi = nc.dram_tensor("Wi", [EMB, G3], F32R, kind="ExternalInput")
    Wf = nc.dram_tensor("Wf", [EMB, G3], F32R, kind="ExternalInput")
    Wr = nc.dram_tensor("Wr", [HID, G3], BF16, kind="ExternalInput")
    bias_b = nc.dram_tensor("bias_b", [128, G3], F32, kind="ExternalInput")
    bias_f = nc.dram_tensor("bias_f", [128, G3], F32, kind="ExternalInput")
    identf = nc.dram_tensor("identf", [128, 128], F32R, kind="ExternalInput")
    out = nc.dram_tensor("out", [BLOC, 2 * HID], F32, kind="ExternalOutput")

    XTd = nc.dram_tensor("XTd", [MT, 128, EMB], F32R)      # internal
    xgd = nc.dram_tensor("xgd", [NTOK, G3], BF16)         # internal

    with tile.TileContext(nc) as tc:
        # ---------------- pass A: gather + transpose ----------------
        with tc.tile_pool(name="pa", bufs=2) as pa, \
             tc.tile_pool(name="pa1", bufs=1) as pa1, \
             tc.tile_pool(name="pa_ps", bufs=4, space="PSUM") as pa_ps:
            ident = pa1.tile([128, 128], F32R)
            nc.sync.dma_start(out=ident[:], in_=identf[:, :])
            for m in range(MT):
                idx_sb = pa.tile([128, 1], I32, tag="idx")
                nc.sync.dma_start(out=idx_sb[:], in_=tok[m * 128:(m + 1) * 128, :])
                x_sb = pa.tile([128, EMB], F32R, tag="x")
                nc.gpsimd.indirect_dma_start(
                    out=x_sb[:], out_offset=None, in_=table[:, :],
                    in_offset=bass.IndirectOffsetOnAxis(ap=idx_sb[:, :1], axis=0))
                xt_sb = pa.tile([128, EMB], F32R, tag="xt")
                for q in range(KT_E):
                    t_ps = pa_ps.tile([128, 128], F32R, space="PSUM", tag="tps")
                    nc.tensor.transpose(out=t_ps[:], in_=x_sb[:, 128 * q:128 * (q + 1)],
                                        identity=ident[:])
                    nc.vector.tensor_copy(xt_sb[:, 128 * q:128 * (q + 1)], t_ps[:])
                nc.sync.dma_start(out=XTd[m, :, :], in_=xt_sb[:])

        tc.strict_bb_all_engine_barrier()
        # ---------------- phase 1: input projection ----------------
        with tc.tile_pool(name="p1w", bufs=2) as p1w, \
             tc.tile_pool(name="p1wf", bufs=1) as p1wf, \
             tc.tile_pool(name="p1", bufs=2) as p1, \
             tc.tile_pool(name="p1s", bufs=1) as p1s, \
             tc.tile_pool(name="p1_ps", bufs=2, space="PSUM") as p1_ps:
            xt0_sb = p1s.tile([128, EMB], F32R)
            nc.sync.dma_start(out=xt0_sb[:], in_=XTd[0, :, :])
            for blk in range(NG):
                cs = slice(GC * blk, GC * (blk + 1))
                wi_sb = p1w.tile([128, KT_E, GC], F32R, tag="wi")
                nc.sync.dma_start(
                    out=wi_sb[:],
                    in_=Wi[:, cs].rearrange("(k p) c -> p k c", p=128))
                wf_sb = p1wf.tile([128, KT_E, GC], F32R, tag="wf")
                nc.sync.dma_start(
                    out=wf_sb[:],
                    in_=Wf[:, cs].rearrange("(k p) c -> p k c", p=128))
                bia_sb = p1.tile([128, GC], F32, tag="bia")
                nc.sync.dma_start(out=bia_sb[:], in_=bias_b[:, cs])
                for m in range(MT):
                    xt_sb = p1.tile([128, EMB], F32R, tag="xtl")
                    nc.sync.dma_start(out=xt_sb[:], in_=XTd[m, :, :])
                    ps = p1_ps.tile([128, GC], F32, space="PSUM", tag="ps")
                    for c in range(3):
                        for k in range(KT_E):
                            nc.tensor.matmul(
                                ps[:, 512 * c:512 * (c + 1)],
                                lhsT=xt_sb[:, 128 * k:128 * (k + 1)],
                                rhs=wi_sb[:, k, 512 * c:512 * (c + 1)],
                                start=(k == 0), stop=(k == KT_E - 1))
                    xg_sb = p1.tile([128, GC], BF16, tag="xg")
                    nc.vector.tensor_add(xg_sb[:], ps[:], bia_sb[:])
                    nc.sync.dma_start(out=xgd[m * 128:(m + 1) * 128, cs], in_=xg_sb[:])
                # forward cell for this block (tokens 0..32 = original last step)
                psf = p1_ps.tile([128, GC], F32, space="PSUM", tag="ps")
                for c in range(3):
                    for k in range(KT_E):
                        nc.tensor.matmul(
                            psf[0:BLOC, 512 * c:512 * (c + 1)],
                            lhsT=xt0_sb[:, 128 * k:128 * k + BLOC],
                            rhs=wf_sb[:, k, 512 * c:512 * (c + 1)],
                            start=(k == 0), stop=(k == KT_E - 1))
                bif_sb = p1s.tile([BLOC, GC], F32, tag="bif")
                nc.sync.dma_start(out=bif_sb[:], in_=bias_f[0:BLOC, cs])
                gf = p1s.tile([BLOC, GC], F32, tag="gf")
                nc.vector.tensor_add(gf[:], psf[0:BLOC, :], bif_sb[:])
                af = p1s.tile([BLOC, HG], F32, tag="af")
                bf = p1s.tile([BLOC, HG], F32, tag="bff")
                cf = p1s.tile([BLOC, HG], F32, tag="cf")
                nc.scalar.activation(af[:], gf[:, 0:HG],
                                     mybir.ActivationFunctionType.Sigmoid)
                nc.scalar.activation(bf[:], gf[:, HG:2 * HG],
                                     mybir.ActivationFunctionType.Tanh)
                nc.scalar.activation(cf[:], gf[:, 2 * HG:3 * HG],
                                     mybir.ActivationFunctionType.Sigmoid)
                nc.vector.tensor_mul(af[:], af[:], bf[:])
                nc.scalar.activation(af[:], af[:],
                                     mybir.ActivationFunctionType.Tanh)
                nc.vector.tensor_mul(af[:], cf[:], af[:])
                nc.sync.dma_start(out=out[:, HG * blk:HG * (blk + 1)], in_=af[:])

        tc.strict_bb_all_engine_barrier()
        # ---------------- phase R: recurrence ----------------
        with tc.tile_pool(name="prw", bufs=1) as prw, \
             tc.tile_pool(name="pr", bufs=2) as pr, \
             tc.tile_pool(name="pr1", bufs=1) as pr1, \
             tc.tile_pool(name="prh", bufs=8) as prh, \
             tc.tile_pool(name="pr_ps", bufs=2, space="PSUM") as pr_ps, \
             tc.tile_pool(name="prt_ps", bufs=2, space="PSUM") as prt_ps:
            wr_sb = prw.tile([128, KT_H, G3], BF16)
            nc.sync.dma_start(
                out=wr_sb[:], in_=Wr[:, :].rearrange("(k p) c -> p k c", p=128))
            identb = pr1.tile([128, 128], BF16)
            make_identity(nc, identb[:])

            a_t = pr1.tile([128, HG], F32)
            b_t = pr1.tile([128, HG], F32)

            def load_xg(s):
                xg_sb = pr.tile([128, GC], BF16, tag="xgs")
                for j in range(NG):
                    nc.sync.dma_start(
                        out=xg_sb[BLOC * j:BLOC * (j + 1), :],
                        in_=xgd[BLOC * s:BLOC * (s + 1), GC * j:GC * (j + 1)])
                return xg_sb

            def act_and_transpose(gi_ap, gg_ap, go_ap, store_out=False):
                """gi/gg/go: [128, HG] gate APs; returns list of 4 hT chunk
                tiles [128,128] (hT[c][:, 32j:32j+32] = k-tile 4j+c)."""
                nc.scalar.activation(a_t[:], gi_ap,
                                     mybir.ActivationFunctionType.Sigmoid)
                nc.scalar.activation(b_t[:], gg_ap,
                                     mybir.ActivationFunctionType.Tanh)
                nc.vector.tensor_mul(a_t[:], a_t[:], b_t[:])      # u = sig(i)*tanh(g)
                nc.scalar.activation(a_t[:], a_t[:],
                                     mybir.ActivationFunctionType.Tanh)  # v
                nc.scalar.activation(b_t[:], go_ap,
                                     mybir.ActivationFunctionType.Sigmoid)  # c
                if store_out:
                    h_t = pr.tile([128, HG], F32, tag="hfin", bufs=1)
                    nc.vector.tensor_mul(h_t[:], b_t[:], a_t[:])
                    for j in range(NG):
                        nc.sync.dma_start(
                            out=out[:, HID + HG * j:HID + HG * (j + 1)],
                            in_=h_t[BLOC * j:BLOC * (j + 1), :])
                    return None
                # chunk-pipelined: mul -> PE transpose -> copy per 128-col chunk
                hTs = []
                for q in range(NG):
                    h_q = pr.tile([128, 128], BF16, tag="h", bufs=3)
                    nc.vector.tensor_mul(h_q[:], b_t[:, 128 * q:128 * (q + 1)],
                                         a_t[:, 128 * q:128 * (q + 1)])
                    t_ps = prt_ps.tile([128, 128], BF16, space="PSUM", tag="tps")
                    nc.tensor.transpose(out=t_ps[:], in_=h_q[:],
                                        identity=identb[:])
                    hT_q = prh.tile([128, 128], BF16, tag="hT")
                    nc.vector.tensor_copy(hT_q[:], t_ps[:])
                    hTs.append(hT_q)
                return hTs

            # step 0: h=0 -> gates are just xg
            xg0 = load_xg(0)
            hT = act_and_transpose(xg0[:, 0:HG], xg0[:, HG:2 * HG],
                                   xg0[:, 2 * HG:3 * HG])

            for s in range(1, n_steps):
                xg_sb = load_xg(s)
                # one PSUM tile per gate bank so banks don't serialize on the
                # DVE adds (Tile psum deps are tile-granular)
                ps_b = []
                for c in range(3):
                    ps_c = pr_ps.tile([128, 512], F32, space="PSUM",
                                      tag=f"gps{c}")
                    for k in range(KT_H):
                        lhs = hT[k % NG][:, BLOC * (k // NG):BLOC * (k // NG) + BLOC]
                        for j in range(NG):
                            nc.tensor.matmul(
                                ps_c[BLOC * j:BLOC * (j + 1), :],
                                lhsT=lhs,
                                rhs=wr_sb[:, k, GC * j + 512 * c:GC * j + 512 * (c + 1)],
                                start=(k == 0), stop=(k == KT_H - 1),
                                tile_position=(0, BLOC * j),
                                skip_group_check=True)
                    # fold xg into this bank as soon as its accumulation is done
                    nc.vector.tensor_add(
                        ps_c[:], ps_c[:], xg_sb[:, 512 * c:512 * (c + 1)])
                    ps_b.append(ps_c)
                hT = act_and_transpose(ps_b[0][:], ps_b[1][:], ps_b[2][:],
                                       store_out=(s == n_steps - 1))
    nc.compile()
    return nc


_BUILT = {}


def _get_built(n_steps=None):
    key = n_steps or N_STEPS
    if key not in _BUILT:
        _BUILT[key] = build(key)
    return _BUILT[key]


def _perm():
    """Row permutation taking PyTorch (i|f|g|o)*2048 rows to 4 groups of
    (i|g|o)*512."""
    p = []
    for j in range(NG):
        for base in (0, 2 * HID, 3 * HID):  # i, g, o blocks
            p.extend(range(base + HG * j, base + HG * (j + 1)))
    return np.array(p)


def prep_inputs(inputs, embed_table, W_ih_f, W_hh_f, b_ih_f, b_hh_f,
                W_ih_b, W_hh_b, b_ih_b, b_hh_b):
    perm = _perm()
    idx = np.asarray(inputs)
    idx = np.where(idx > VOCAB, 0, idx).astype(np.int64)
    idx = np.clip(idx, 0, VOCAB - 1).astype(np.int32)

    Wi_p = np.ascontiguousarray(np.asarray(W_ih_b)[perm].T.astype(np.float32))
    Wf_p = np.ascontiguousarray(np.asarray(W_ih_f)[perm].T.astype(np.float32))
    Wr_p = np.ascontiguousarray(
        np.asarray(W_hh_b)[perm].T.astype(ml_dtypes.bfloat16))
    bb = (np.asarray(b_ih_b) + np.asarray(b_hh_b))[perm].astype(np.float32)
    bf = (np.asarray(b_ih_f) + np.asarray(b_hh_f))[perm].astype(np.float32)
    bias_b_t = np.ascontiguousarray(np.broadcast_to(bb, (128, G3)))
    bias_f_t = np.ascontiguousarray(np.broadcast_to(bf, (128, G3)))
    table = np.ascontiguousarray(np.asarray(embed_table, dtype=np.float32))
    identf = np.eye(128, dtype=np.float32)

    in_maps = []
    for c in range(NCORES):
        sl = idx[BLOC * c:BLOC * (c + 1)]          # [32, 128]
        tok = np.ascontiguousarray(sl[:, ::-1].T.reshape(NTOK, 1))  # t-major rev
        in_maps.append({
            "tok": tok, "table": table, "Wi": Wi_p, "Wf": Wf_p, "Wr": Wr_p,
            "bias_b": bias_b_t, "bias_f": bias_f_t, "identf": identf,
        })
    return in_maps


def kernel(**inputs) -> np.ndarray:
    from concourse.bass_utils import run_bass_kernel_spmd
    nc = _get_built()
    in_maps = prep_inputs(**inputs)
    res = run_bass_kernel_spmd(nc, in_maps, core_ids=list(range(NCORES)))
    return np.concatenate([res.results[c]["out"] for c in range(NCORES)], axis=0)



# revision 16
# speedup vs baseline: 1.0625x; 1.0625x over previous
"""Bidirectional-LSTM (degenerate variant) Trainium2 kernel, v2.

Reference semantics: forward direction only uses the last timestep (h/c never
update), backward direction is an h-only recurrence (c stays zero), so only
the i/g/o gates matter:

    h = sig(o) * tanh(sig(i) * tanh(g))
    fwd: gates = x_last @ W_ih_f.T + b_f
    bwd: scan t = S-1..0, gates = x_t @ W_ih_b.T + b_b + h @ W_hh_b.T

Distribution: data-parallel over batch (32 rows/core x 8 cores), weights
replicated.  All matmul operands fp16 (measured end-to-end rel err ~1e-3).

Per core:
  phase 1 (fused): per 128-token m-tile: embedding gather (indirect DMA,
    fp16 table) -> 8x XBAR DMA-transpose (off the PE) -> input projection
    with Wi fully SBUF-resident; k-outer/band-inner loops + M=64 A/B column
    split so LDWEIGHTS overlaps streaming.  Forward cell folded into the
    last 3 m-tiles (quadrant-packed, Wf streamed in 8KB chunks).
  phase R: 128-step recurrence.  gates = Wr.T @ h with 4-way column-tiled
    M=32 matmuls; banks i,g interleaved per k-slot (shared stationary),
    o-bank last so the sig/tanh chain hides under its matmuls; bias+xg
    folded in as identity matmuls on the PE; per-128-col-chunk
    sig(o) -> mul -> XBAR DMA-transpose pipeline produces the next step's
    stationary hT without touching the PE.

Gate columns are host-permuted into 4 groups of (i|g|o) x 512 hid dims so
PSUM column-group j directly yields h[:, 512j:512j+512].
"""

import numpy as np

import concourse.bass as bass
import concourse.bacc as bacc
import concourse.mybir as mybir
import concourse.tile as tile
from concourse.masks import make_identity

VOCAB, EMB, HID = 50000, 1024, 2048
BATCH, SEQ = 256, 128
NCORES = 8
BLOC = BATCH // NCORES            # 32 batch rows per core
NTOK = BLOC * SEQ                 # 4096 tokens per core
NG = 4                            # gate column groups (= hid groups)
GC = 3 * HID // NG                # 1536 gate cols per group (i|g|o x 512)
HG = HID // NG                    # 512 hid dims per group
G3 = 3 * HID                      # 6144 total igo gate cols
MT = NTOK // 128                  # 32 token m-tiles
KT_E = EMB // 128                 # 8 k-tiles, input projection
KT_H = HID // 128                 # 16 k-tiles, recurrence
NBAND = G3 // 512                 # 12 phase-1 column bands

F32 = mybir.dt.float32
F16 = mybir.dt.float16
I32 = mybir.dt.int32

N_STEPS = SEQ
# k order so hT chunk q (holding k-tiles {q,4+q,8+q,12+q}) is consumed
# chunk-major: the first 4 slots need only chunk 0, etc.
K_ORDER = [0, 4, 8, 12, 1, 5, 9, 13, 2, 6, 10, 14, 3, 7, 11, 15]
USE_DMA_TR = True                 # XBAR DMA transpose vs PE transpose


def build(n_steps=None):
    n_steps = n_steps or N_STEPS
    nc = bacc.Bacc("TRN2", target_bir_lowering=False, debug=False,
                   num_devices=NCORES)

    tok = nc.dram_tensor("tok", [NTOK, 1], I32, kind="ExternalInput")
    table = nc.dram_tensor("table", [VOCAB, EMB], F16, kind="ExternalInput")
    Wi = nc.dram_tensor("Wi", [EMB, G3], F16, kind="ExternalInput")
    Wf = nc.dram_tensor("Wf", [EMB, G3], F16, kind="ExternalInput")
    Wr = nc.dram_tensor("Wr", [HID, G3], F16, kind="ExternalInput")
    bias_b = nc.dram_tensor("bias_b", [128, G3], F32, kind="ExternalInput")
    # forward bias packed per (c, j): [3, 128, 512], partition 32j+b -> group j
    bias_f = nc.dram_tensor("bias_f", [3, 128, 512], F32, kind="ExternalInput")
    out = nc.dram_tensor("out", [BLOC, 2 * HID], F32, kind="ExternalOutput")

    xgd = nc.dram_tensor("xgd", [NTOK, G3], F16)          # internal

    with tile.TileContext(nc) as tc:
        with tc.tile_pool(name="pk", bufs=1) as pk:
            ident = pk.tile([128, 128], F16)
            make_identity(nc, ident[:])

            # ======== phase 1: gather + transpose + input projection ========
            with tc.tile_pool(name="p1w", bufs=1) as p1w, \
                 tc.tile_pool(name="p1x", bufs=3) as p1x, \
                 tc.tile_pool(name="p1g", bufs=2) as p1g, \
                 tc.tile_pool(name="p1o", bufs=6) as p1o, \
                 tc.tile_pool(name="p1f", bufs=1) as p1f, \
                 tc.tile_pool(name="p1_ps", bufs=1, space="PSUM") as p1_ps, \
                 tc.tile_pool(name="pf_ps", bufs=1, space="PSUM") as pf_ps:
                wi_sb = p1w.tile([128, KT_E, G3], F16)
                nc.sync.dma_start(
                    out=wi_sb[:], in_=Wi[:, :].rearrange("(k p) c -> p k c", p=128))
                bia_sb = p1w.tile([128, G3], F32, tag="bia")
                nc.sync.dma_start(out=bia_sb[:], in_=bias_b[:, :])
                xt0_sb = p1f.tile([128, KT_E, 128], F16, tag="xt0")

                gf_c = []      # forward gate banks [128, 512] f32
                for m in range(MT):
                    idx_sb = p1g.tile([128, 1], I32, tag="idx")
                    nc.sync.dma_start(out=idx_sb[:],
                                      in_=tok[m * 128:(m + 1) * 128, :])
                    x_sb = p1g.tile([128, EMB], F16, tag="x")
                    nc.gpsimd.indirect_dma_start(
                        out=x_sb[:], out_offset=None, in_=table[:, :],
                        in_offset=bass.IndirectOffsetOnAxis(ap=idx_sb[:, :1], axis=0))
                    xt_sb = p1x.tile([128, KT_E, 128], F16, tag="xt")
                    for q in range(KT_E):
                        if USE_DMA_TR:
                            nc.sync.dma_start_transpose(
                                out=xt_sb[:, q, :],
                                in_=x_sb[:, 128 * q:128 * (q + 1)])
                        else:
                            t_ps = p1_ps.tile([128, 128], F16, space="PSUM",
                                              tag="tps", bufs=2)
                            nc.tensor.transpose(
                                out=t_ps[:], in_=x_sb[:, 128 * q:128 * (q + 1)],
                                identity=ident[:])
                            nc.vector.tensor_copy(xt_sb[:, q, :], t_ps[:])
                    if m == 0:
                        # keep m-tile 0 transposed for the forward cell
                        nc.vector.tensor_copy(xt0_sb[:], xt_sb[:])

                    for bh in range(2):
                        ps_b = []
                        for b in range(6):
                            ps = p1_ps.tile([128, 512], F32, space="PSUM",
                                            tag=f"ps{b}")
                            ps_b.append(ps)
                        for k in range(KT_E):
                            # 4-way M=32 col tiling: the four 27ns LDWEIGHTS
                            # run concurrently, vs one serial 107ns full-width
                            for b in range(6):
                                cs = 3072 * bh + 512 * b
                                for q in range(NG):
                                    nc.tensor.matmul(
                                        ps_b[b][32 * q:32 * (q + 1), :],
                                        lhsT=xt_sb[:, k, 32 * q:32 * (q + 1)],
                                        rhs=wi_sb[:, k, cs:cs + 512],
                                        start=(k == 0), stop=(k == KT_E - 1),
                                        tile_position=(0, 32 * q),
                                        skip_group_check=True)
                        for b in range(6):
                            cs = 3072 * bh + 512 * b
                            xg_sb = p1o.tile([128, 512], F16, tag="xg")
                            nc.vector.tensor_add(xg_sb[:], ps_b[b][:],
                                                 bia_sb[:, cs:cs + 512])
                            nc.scalar.dma_start(
                                out=xgd[m * 128:(m + 1) * 128, cs:cs + 512],
                                in_=xg_sb[:])

                    # forward cell: gate bank c at m-tiles 29/30/31
                    if m >= MT - 3:
                        c = m - (MT - 3)
                        wf_js = []
                        for j in range(NG):
                            wf_j = p1f.tile([128, KT_E, 512], F16, tag=f"wf{j}")
                            nc.sync.dma_start(
                                out=wf_j[:],
                                in_=Wf[:, GC * j + 512 * c:GC * j + 512 * (c + 1)]
                                .rearrange("(k p) c -> p k c", p=128))
                            wf_js.append(wf_j)
                        psf = pf_ps.tile([128, 512], F32, space="PSUM",
                                         tag="psf")
                        for k in range(KT_E):
                            lhs = xt0_sb[:, k, 0:BLOC]
                            for j in range(NG):
                                nc.tensor.matmul(
                                    psf[BLOC * j:BLOC * (j + 1), :],
                                    lhsT=lhs, rhs=wf_js[j][:, k, :],
                                    start=(k == 0), stop=(k == KT_E - 1),
                                    tile_position=(0, BLOC * j),
                                    skip_group_check=True)
                        bf_sb = p1f.tile([128, 512], F32, tag=f"bf{c}")
                        nc.sync.dma_start(out=bf_sb[:], in_=bias_f[c, :, :])
                        gf = p1f.tile([128, 512], F32, tag=f"gf{c}")
                        nc.vector.tensor_add(gf[:], psf[:], bf_sb[:])
                        gf_c.append(gf)

                # forward activations: h_f = sig(o)*tanh(sig(i)*tanh(g))
                af = p1f.tile([128, 512], F16, tag="af")
                bf2 = p1f.tile([128, 512], F16, tag="bff")
                nc.scalar.activation(af[:], gf_c[0][:],
                                     mybir.ActivationFunctionType.Sigmoid)
                nc.scalar.activation(bf2[:], gf_c[1][:],
                                     mybir.ActivationFunctionType.Tanh)
                nc.vector.tensor_mul(af[:], af[:], bf2[:])
                nc.scalar.activation(af[:], af[:],
                                     mybir.ActivationFunctionType.Tanh)
                nc.scalar.activation(bf2[:], gf_c[2][:],
                                     mybir.ActivationFunctionType.Sigmoid)
                hf = p1f.tile([128, 512], F32, tag="hf")
                nc.vector.tensor_mul(hf[:], bf2[:], af[:])
                for j in range(NG):
                    nc.sync.dma_start(
                        out=out[:, HG * j:HG * (j + 1)],
                        in_=hf[BLOC * j:BLOC * (j + 1), :])

            tc.strict_bb_all_engine_barrier()
            # ======== phase R: recurrence ========
            with tc.tile_pool(name="prw", bufs=1) as prw, \
                 tc.tile_pool(name="prx", bufs=2) as prx, \
                 tc.tile_pool(name="pra", bufs=1) as pra, \
                 tc.tile_pool(name="prh", bufs=4) as prh, \
                 tc.tile_pool(name="prt", bufs=8) as prt, \
                 tc.tile_pool(name="pr_ps", bufs=1, space="PSUM") as pr_ps, \
                 tc.tile_pool(name="prt_ps", bufs=2, space="PSUM") as prt_ps:
                wr_sb = prw.tile([128, KT_H, G3], F16)
                # split the 24MB Wr load across both HW DMA queues
                nc.sync.dma_start(
                    out=wr_sb[:, 0:KT_H // 2, :],
                    in_=Wr[0:HID // 2, :].rearrange("(k p) c -> p k c", p=128))
                nc.scalar.dma_start(
                    out=wr_sb[:, KT_H // 2:KT_H, :],
                    in_=Wr[HID // 2:HID, :].rearrange("(k p) c -> p k c", p=128))

                a_t = pra.tile([128, HG], F16)
                b_t = pra.tile([128, HG], F16)

                def load_xg(s):
                    xg_sb = prx.tile([128, GC], F16, tag="xgs")
                    for j in range(NG):
                        nc.sync.dma_start(
                            out=xg_sb[BLOC * j:BLOC * (j + 1), :],
                            in_=xgd[BLOC * s:BLOC * (s + 1),
                                    GC * j:GC * (j + 1)])
                    return xg_sb

                xg_tiles = {s: load_xg(s) for s in range(min(2, n_steps))}

                def act_head(gi_ap, gg_ap):
                    """a_t = tanh(sig(i) * tanh(g)); runs under bank o's
                    matmuls.  Must be emitted before bank o's DVE adds so the
                    mul isn't stuck behind them in the DVE FIFO."""
                    nc.scalar.activation(a_t[:], gi_ap,
                                         mybir.ActivationFunctionType.Sigmoid)
                    nc.scalar.activation(b_t[:], gg_ap,
                                         mybir.ActivationFunctionType.Tanh)
                    nc.vector.tensor_mul(a_t[:], a_t[:], b_t[:])
                    nc.scalar.activation(a_t[:], a_t[:],
                                         mybir.ActivationFunctionType.Tanh)

                def act_tail(go_tile, go_off, store_out):
                    """h = sig(o) * a_t, transposed into 4 hT chunks, or the
                    final h stored.  go = go_tile[:, go_off:go_off+HG]."""
                    if store_out:
                        nc.scalar.activation(
                            b_t[:], go_tile[:, go_off:go_off + HG],
                            mybir.ActivationFunctionType.Sigmoid)
                        h_t = pra.tile([128, HG], F32, tag="hfin")
                        nc.vector.tensor_mul(h_t[:], b_t[:], a_t[:])
                        for j in range(NG):
                            nc.sync.dma_start(
                                out=out[:, HID + HG * j:HID + HG * (j + 1)],
                                in_=h_t[BLOC * j:BLOC * (j + 1), :])
                        return None
                    hTs = []
                    for q in range(NG):
                        # per-chunk: sig(o) -> mul -> transpose
                        nc.scalar.activation(
                            b_t[:, 128 * q:128 * (q + 1)],
                            go_tile[:, go_off + 128 * q:go_off + 128 * (q + 1)],
                            mybir.ActivationFunctionType.Sigmoid)
                        h_q = prh.tile([128, 128], F16, tag="h")
                        nc.vector.tensor_mul(h_q[:],
                                             b_t[:, 128 * q:128 * (q + 1)],
                                             a_t[:, 128 * q:128 * (q + 1)])
                        hT_q = prt.tile([128, 128], F16, tag="hT")
                        if USE_DMA_TR:
                            nc.sync.dma_start_transpose(out=hT_q[:], in_=h_q[:])
                        else:
                            t_ps = prt_ps.tile([128, 128], F16, space="PSUM",
                                               tag="tps")
                            nc.tensor.transpose(out=t_ps[:], in_=h_q[:],
                                                identity=ident[:])
                            nc.vector.tensor_copy(hT_q[:], t_ps[:])
                        hTs.append(hT_q)
                    return hTs

                # step 0: h = 0 -> gates are just xg
                xg0 = xg_tiles[0]
                act_head(xg0[:, 0:HG], xg0[:, HG:2 * HG])
                hT = act_tail(xg0, 2 * HG, store_out=(n_steps == 1))

                for s in range(1, n_steps):
                    xg_sb = xg_tiles.pop(s)
                    if s + 1 < n_steps:
                        xg_tiles[s + 1] = load_xg(s + 1)
                    ps_b = []
                    for c in range(3):
                        gps = pr_ps.tile([128, 512], F32, space="PSUM",
                                         tag=f"gps{c}")
                        ps_b.append(gps)
                    # bank-major: i and g early so the sig/tanh chain can run
                    # under bank o's matmuls; K_ORDER consumes hT chunk-major.
                    # xg is folded in as an identity matmul opening each
                    # bank's accumulation (start=True) -> no DVE adds, and the
                    # activations chain directly off the matmul stop.
                    def bank_mms(c):
                        nc.tensor.matmul(
                            ps_b[c][:], lhsT=ident[:],
                            rhs=xg_sb[:, 512 * c:512 * (c + 1)],
                            start=True, stop=False, skip_group_check=True)
                        for ki, k in enumerate(K_ORDER):
                            lhs = hT[k % NG][:, BLOC * (k // NG):
                                             BLOC * (k // NG) + BLOC]
                            for j in range(NG):
                                nc.tensor.matmul(
                                    ps_b[c][BLOC * j:BLOC * (j + 1), :],
                                    lhsT=lhs,
                                    rhs=wr_sb[:, k,
                                              GC * j + 512 * c:
                                              GC * j + 512 * (c + 1)],
                                    start=False, stop=(ki == KT_H - 1),
                                    tile_position=(0, BLOC * j),
                                    skip_group_check=True)

                    bank_mms(0)
                    bank_mms(1)
                    act_head(ps_b[0][:], ps_b[1][:])
                    bank_mms(2)
                    hT = act_tail(ps_b[2], 0,
                                  store_out=(s == n_steps - 1))
    nc.compile()
    return nc


_BUILT = {}


def _get_built(n_steps=None):
    key = n_steps or N_STEPS
    if key not in _BUILT:
        _BUILT[key] = build(key)
    return _BUILT[key]


def _perm():
    """Row permutation taking PyTorch (i|f|g|o)*2048 rows to 4 groups of
    (i|g|o)*512."""
    p = []
    for j in range(NG):
        for base in (0, 2 * HID, 3 * HID):  # i, g, o blocks
            p.extend(range(base + HG * j, base + HG * (j + 1)))
    return np.array(p)


def prep_inputs(inputs, embed_table, W_ih_f, W_hh_f, b_ih_f, b_hh_f,
                W_ih_b, W_hh_b, b_ih_b, b_hh_b):
    perm = _perm()
    idx = np.asarray(inputs)
    idx = np.where(idx > VOCAB, 0, idx).astype(np.int64)
    idx = np.clip(idx, 0, VOCAB - 1).astype(np.int32)

    Wi_p = np.ascontiguousarray(np.asarray(W_ih_b)[perm].T.astype(np.float16))
    Wf_p = np.ascontiguousarray(np.asarray(W_ih_f)[perm].T.astype(np.float16))
    Wr_p = np.ascontiguousarray(np.asarray(W_hh_b)[perm].T.astype(np.float16))
    bb = (np.asarray(b_ih_b) + np.asarray(b_hh_b))[perm].astype(np.float32)
    bf = (np.asarray(b_ih_f) + np.asarray(b_hh_f))[perm].astype(np.float32)
    bias_b_t = np.ascontiguousarray(np.broadcast_to(bb, (128, G3)))
    # bias_f packed [3, 128, 512]: partition 32j+b -> gate (c, group j)
    bias_f_t = np.empty((3, 128, 512), np.float32)
    for c in range(3):
        for j in range(NG):
            bias_f_t[c, BLOC * j:BLOC * (j + 1), :] = \
                bf[GC * j + 512 * c:GC * j + 512 * (c + 1)]
    table = np.ascontiguousarray(np.asarray(embed_table).astype(np.float16))

    in_maps = []
    for c in range(NCORES):
        sl = idx[BLOC * c:BLOC * (c + 1)]          # [32, 128]
        tok = np.ascontiguousarray(sl[:, ::-1].T.reshape(NTOK, 1))  # t-major rev
        in_maps.append({
            "tok": tok, "table": table, "Wi": Wi_p, "Wf": Wf_p, "Wr": Wr_p,
            "bias_b": bias_b_t, "bias_f": bias_f_t,
        })
    return in_maps


def kernel(**inputs) -> np.ndarray:
    from concourse.bass_utils import run_bass_kernel_spmd
    nc = _get_built()
    in_maps = prep_inputs(**inputs)
    res = run_bass_kernel_spmd(nc, in_maps, core_ids=list(range(NCORES)))
    return np.concatenate([res.results[c]["out"] for c in range(NCORES)], axis=0)


# revision 19
# speedup vs baseline: 1.0957x; 1.0313x over previous
"""Bidirectional-LSTM (degenerate variant) Trainium2 kernel, v3.

Reference semantics: forward direction only uses the last timestep (h/c never
update), backward direction is an h-only recurrence (c stays zero), so only
the i/g/o gates matter:

    h = sig(o) * tanh(sig(i) * tanh(g))
    fwd: gates = x_last @ W_ih_f.T + b_f
    bwd: scan t = S-1..0, gates = x_t @ W_ih_b.T + b_b + h @ W_hh_b.T

Distribution: data-parallel over batch (32 rows/core x 8 cores), weights
replicated.  All matmul operands fp16 (measured end-to-end rel err ~1e-3).

Per core:
  phase 1 (fused): per 128-token m-tile: embedding gather (indirect DMA, fp16
    table) -> one batched XBAR DMA-transpose ([128,1024] -> [128,8,128], off
    the PE) -> input projection with Wi SBUF-resident (two half tiles loaded
    on both HW DMA queues); 4-way M=32 col-tiled matmuls so the four 27ns
    LDWEIGHTS run concurrently instead of one serial 107ns load.  DMAs are
    batched (1 store per gate half) to avoid completion-semaphore convoys.
    Forward cell folded into the last 3 m-tiles (quadrant-packed, Wf
    streamed).
  phase R: 128-step recurrence.  gates = Wr.T @ h, 4-way col-tiled M=32;
    banks i,g first so the sig/tanh chain hides under bank o's matmuls;
    xg+bias folded in as an identity matmul opening each bank's PSUM
    accumulation (no DVE adds; next step's i/g identity matmuls are emitted
    early to fill the inter-step PE gap and keep HAM warm); per-128-col-chunk
    sig(o) -> mul -> PE transpose -> copy pipeline rebuilds the stationary hT
    with K_ORDER consuming chunks in completion order.

Gate columns are host-permuted into 4 groups of (i|g|o) x 512 hid dims so
PSUM column-group j directly yields h[:, 512j:512j+512].
"""

import numpy as np

import concourse.bass as bass
import concourse.bacc as bacc
import concourse.mybir as mybir
import concourse.tile as tile
from concourse.masks import make_identity

VOCAB, EMB, HID = 50000, 1024, 2048
BATCH, SEQ = 256, 128
NCORES = 8
BLOC = BATCH // NCORES            # 32 batch rows per core
NTOK = BLOC * SEQ                 # 4096 tokens per core
NG = 4                            # gate column groups (= hid groups)
GC = 3 * HID // NG                # 1536 gate cols per group (i|g|o x 512)
HG = HID // NG                    # 512 hid dims per group
G3 = 3 * HID                      # 6144 total igo gate cols
MT = NTOK // 128                  # 32 token m-tiles
KT_E = EMB // 128                 # 8 k-tiles, input projection
KT_H = HID // 128                 # 16 k-tiles, recurrence

F32 = mybir.dt.float32
F16 = mybir.dt.float16
I32 = mybir.dt.int32

N_STEPS = SEQ
# k order so hT chunk q (holding k-tiles {q,4+q,8+q,12+q}) is consumed
# chunk-major: the first 4 slots need only chunk 0, etc.
K_ORDER = [0, 4, 8, 12, 1, 5, 9, 13, 2, 6, 10, 14, 3, 7, 11, 15]


def build(n_steps=None):
    n_steps = n_steps or N_STEPS
    nc = bacc.Bacc("TRN2", target_bir_lowering=False, debug=False,
                   num_devices=NCORES)

    tok = nc.dram_tensor("tok", [NTOK, 1], I32, kind="ExternalInput")
    table = nc.dram_tensor("table", [VOCAB, EMB], F16, kind="ExternalInput")
    Wi = nc.dram_tensor("Wi", [EMB, G3], F16, kind="ExternalInput")
    Wf = nc.dram_tensor("Wf", [EMB, G3], F16, kind="ExternalInput")
    Wr = nc.dram_tensor("Wr", [HID, G3], F16, kind="ExternalInput")
    bias_b = nc.dram_tensor("bias_b", [128, G3], F32, kind="ExternalInput")
    # forward bias packed per (c, j): [3, 128, 512], partition 32j+b -> group j
    bias_f = nc.dram_tensor("bias_f", [3, 128, 512], F32, kind="ExternalInput")
    out = nc.dram_tensor("out", [BLOC, 2 * HID], F32, kind="ExternalOutput")

    xgd = nc.dram_tensor("xgd", [NTOK, G3], F16)          # internal

    with tile.TileContext(nc) as tc:
        with tc.tile_pool(name="pk", bufs=1) as pk:
            ident = pk.tile([128, 128], F16)
            make_identity(nc, ident[:])

            # ======== phase 1: gather + transpose + input projection ========
            with tc.tile_pool(name="p1w", bufs=1) as p1w, \
                 tc.tile_pool(name="p1x", bufs=3) as p1x, \
                 tc.tile_pool(name="p1g", bufs=2) as p1g, \
                 tc.tile_pool(name="p1o", bufs=3) as p1o, \
                 tc.tile_pool(name="p1f", bufs=1) as p1f, \
                 tc.tile_pool(name="p1_ps", bufs=1, space="PSUM") as p1_ps, \
                 tc.tile_pool(name="pf_ps", bufs=1, space="PSUM") as pf_ps:
                # Wi halves on both HW DMA queues so bh0 matmuls start early
                wi_h = []
                for bh in range(2):
                    wi_t = p1w.tile([128, KT_E, 3072], F16, tag=f"wi{bh}")
                    eng = nc.sync if bh == 0 else nc.scalar
                    eng.dma_start(
                        out=wi_t[:],
                        in_=Wi[:, 3072 * bh:3072 * (bh + 1)]
                        .rearrange("(k p) c -> p k c", p=128))
                    wi_h.append(wi_t)
                bia_sb = p1w.tile([128, G3], F32, tag="bia")
                nc.sync.dma_start(out=bia_sb[:], in_=bias_b[:, :])
                xt0_sb = p1f.tile([128, KT_E, 128], F16, tag="xt0")

                gf_c = []      # forward gate banks [128, 512] f32
                for m in range(MT):
                    idx_sb = p1g.tile([128, 1], I32, tag="idx")
                    nc.sync.dma_start(out=idx_sb[:],
                                      in_=tok[m * 128:(m + 1) * 128, :])
                    x_sb = p1g.tile([128, EMB], F16, tag="x")
                    nc.gpsimd.indirect_dma_start(
                        out=x_sb[:], out_offset=None, in_=table[:, :],
                        in_offset=bass.IndirectOffsetOnAxis(ap=idx_sb[:, :1], axis=0))
                    # one batched XBAR transpose: xt[p, k, t] = x[t, 128k+p]
                    xt_sb = p1x.tile([128, KT_E, 128], F16, tag="xt")
                    nc.sync.dma_start_transpose(out=xt_sb[:], in_=x_sb[:])
                    if m == 0:
                        # keep m-tile 0 transposed for the forward cell
                        nc.vector.tensor_copy(xt0_sb[:], xt_sb[:])

                    for bh in range(2):
                        xg_sb = p1o.tile([128, 3072], F16, tag="xg")
                        ps_b = []
                        for b in range(6):
                            ps = p1_ps.tile([128, 512], F32, space="PSUM",
                                            tag=f"ps{b}")
                            ps_b.append(ps)
                        for k in range(KT_E):
                            # 4-way M=32 col tiling: four 27ns LDWEIGHTS run
                            # concurrently, vs one serial 107ns full-width
                            for b in range(6):
                                for q in range(NG):
                                    nc.tensor.matmul(
                                        ps_b[b][32 * q:32 * (q + 1), :],
                                        lhsT=xt_sb[:, k, 32 * q:32 * (q + 1)],
                                        rhs=wi_h[bh][:, k, 512 * b:512 * (b + 1)],
                                        start=(k == 0), stop=(k == KT_E - 1),
                                        tile_position=(0, 32 * q),
                                        skip_group_check=True)
                        for b in range(6):
                            cs = 3072 * bh + 512 * b
                            nc.vector.tensor_add(
                                xg_sb[:, 512 * b:512 * (b + 1)], ps_b[b][:],
                                bia_sb[:, cs:cs + 512])
                        nc.scalar.dma_start(
                            out=xgd[m * 128:(m + 1) * 128,
                                    3072 * bh:3072 * (bh + 1)],
                            in_=xg_sb[:])

                    # forward cell: gate bank c at m-tiles 29/30/31
                    if m >= MT - 3:
                        c = m - (MT - 3)
                        wf_js = []
                        for j in range(NG):
                            wf_j = p1f.tile([128, KT_E, 512], F16, tag=f"wf{j}")
                            nc.sync.dma_start(
                                out=wf_j[:],
                                in_=Wf[:, GC * j + 512 * c:GC * j + 512 * (c + 1)]
                                .rearrange("(k p) c -> p k c", p=128))
                            wf_js.append(wf_j)
                        psf = pf_ps.tile([128, 512], F32, space="PSUM",
                                         tag="psf")
                        for k in range(KT_E):
                            lhs = xt0_sb[:, k, 0:BLOC]
                            for j in range(NG):
                                nc.tensor.matmul(
                                    psf[BLOC * j:BLOC * (j + 1), :],
                                    lhsT=lhs, rhs=wf_js[j][:, k, :],
                                    start=(k == 0), stop=(k == KT_E - 1),
                                    tile_position=(0, BLOC * j),
                                    skip_group_check=True)
                        bf_sb = p1f.tile([128, 512], F32, tag=f"bf{c}")
                        nc.sync.dma_start(out=bf_sb[:], in_=bias_f[c, :, :])
                        gf = p1f.tile([128, 512], F32, tag=f"gf{c}")
                        nc.vector.tensor_add(gf[:], psf[:], bf_sb[:])
                        gf_c.append(gf)

                # forward activations: h_f = sig(o)*tanh(sig(i)*tanh(g))
                af = p1f.tile([128, 512], F16, tag="af")
                bf2 = p1f.tile([128, 512], F16, tag="bff")
                nc.scalar.activation(af[:], gf_c[0][:],
                                     mybir.ActivationFunctionType.Sigmoid)
                nc.scalar.activation(bf2[:], gf_c[1][:],
                                     mybir.ActivationFunctionType.Tanh)
                nc.vector.tensor_mul(af[:], af[:], bf2[:])
                nc.scalar.activation(af[:], af[:],
                                     mybir.ActivationFunctionType.Tanh)
                nc.scalar.activation(bf2[:], gf_c[2][:],
                                     mybir.ActivationFunctionType.Sigmoid)
                hf = p1f.tile([128, 512], F32, tag="hf")
                nc.vector.tensor_mul(hf[:], bf2[:], af[:])
                for j in range(NG):
                    nc.sync.dma_start(
                        out=out[:, HG * j:HG * (j + 1)],
                        in_=hf[BLOC * j:BLOC * (j + 1), :])

            tc.strict_bb_all_engine_barrier()
            # ======== phase R: recurrence ========
            with tc.tile_pool(name="prw", bufs=1) as prw, \
                 tc.tile_pool(name="prx", bufs=2) as prx, \
                 tc.tile_pool(name="pra", bufs=1) as pra, \
                 tc.tile_pool(name="prh", bufs=4) as prh, \
                 tc.tile_pool(name="prt", bufs=8) as prt, \
                 tc.tile_pool(name="pr_ps", bufs=1, space="PSUM") as pr_ps, \
                 tc.tile_pool(name="prt_ps", bufs=2, space="PSUM") as prt_ps:
                # Wr split: [i|g] cols (needed first) and [o] cols, each
                # loaded as two k-halves on both HW DMA queues
                wr01 = prw.tile([128, KT_H, NG, 1024], F16, tag="wr01")
                wr2 = prw.tile([128, KT_H, NG, 512], F16, tag="wr2")
                wr_v = Wr[:, :].rearrange("(k p) (j c) -> p k j c",
                                          p=128, j=NG)
                kh = KT_H // 2
                for j in range(NG):
                    nc.sync.dma_start(out=wr01[:, 0:kh, j, :],
                                      in_=wr_v[:, 0:kh, j, 0:1024])
                    nc.scalar.dma_start(out=wr01[:, kh:KT_H, j, :],
                                        in_=wr_v[:, kh:KT_H, j, 0:1024])
                for j in range(NG):
                    nc.sync.dma_start(out=wr2[:, 0:kh, j, :],
                                      in_=wr_v[:, 0:kh, j, 1024:1536])
                    nc.scalar.dma_start(out=wr2[:, kh:KT_H, j, :],
                                        in_=wr_v[:, kh:KT_H, j, 1024:1536])

                a_t = pra.tile([128, HG], F16)
                b_t = pra.tile([128, HG], F16)

                def load_xg(s):
                    # partition 32j+b <- xgd[32s+b, 1536j:1536(j+1)]
                    xg_sb = prx.tile([128, GC], F16, tag="xgs")
                    for j in range(NG):
                        nc.sync.dma_start(
                            out=xg_sb[BLOC * j:BLOC * (j + 1), :],
                            in_=xgd[BLOC * s:BLOC * (s + 1),
                                    GC * j:GC * (j + 1)])
                    return xg_sb

                xg_tiles = {s: load_xg(s) for s in range(min(2, n_steps))}

                def ident_add(ps, xg_sb, c):
                    """Open bank c's PSUM accumulation with ps = xg (identity
                    matmul); needs only xg, so next step's i/g adds fill the
                    inter-step PE gap."""
                    nc.tensor.matmul(
                        ps[:], lhsT=ident[:],
                        rhs=xg_sb[:, 512 * c:512 * (c + 1)],
                        start=True, stop=False, skip_group_check=True)

                def bank_k_mms(ps, c, hT):
                    wr_t = wr01 if c < 2 else wr2
                    co = 512 * c if c < 2 else 0
                    for ki, k in enumerate(K_ORDER):
                        lhs = hT[k % NG][:, BLOC * (k // NG):
                                         BLOC * (k // NG) + BLOC]
                        for j in range(NG):
                            nc.tensor.matmul(
                                ps[BLOC * j:BLOC * (j + 1), :],
                                lhsT=lhs,
                                rhs=wr_t[:, k, j, co:co + 512],
                                start=False, stop=(ki == KT_H - 1),
                                tile_position=(0, BLOC * j),
                                skip_group_check=True)

                def act_head(gi_ap, gg_ap):
                    """a_t = tanh(sig(i) * tanh(g)); runs under bank o's
                    matmuls."""
                    nc.scalar.activation(a_t[:], gi_ap,
                                         mybir.ActivationFunctionType.Sigmoid)
                    nc.scalar.activation(b_t[:], gg_ap,
                                         mybir.ActivationFunctionType.Tanh)
                    nc.vector.tensor_mul(a_t[:], a_t[:], b_t[:])
                    nc.scalar.activation(a_t[:], a_t[:],
                                         mybir.ActivationFunctionType.Tanh)

                def act_tail(go_tile, go_off, store_out):
                    """h = sig(o) * a_t, per-chunk, PE-transposed into 4 hT
                    chunks (kept on the PE: low latency + keeps HAM warm); or
                    the final h stored."""
                    if store_out:
                        nc.scalar.activation(
                            b_t[:], go_tile[:, go_off:go_off + HG],
                            mybir.ActivationFunctionType.Sigmoid)
                        h_t = pra.tile([128, HG], F32, tag="hfin")
                        nc.vector.tensor_mul(h_t[:], b_t[:], a_t[:])
                        for j in range(NG):
                            nc.sync.dma_start(
                                out=out[:, HID + HG * j:HID + HG * (j + 1)],
                                in_=h_t[BLOC * j:BLOC * (j + 1), :])
                        return None
                    hTs = []
                    for q in range(NG):
                        nc.scalar.activation(
                            b_t[:, 128 * q:128 * (q + 1)],
                            go_tile[:, go_off + 128 * q:go_off + 128 * (q + 1)],
                            mybir.ActivationFunctionType.Sigmoid)
                        h_q = prh.tile([128, 128], F16, tag="h")
                        nc.vector.tensor_mul(h_q[:],
                                             b_t[:, 128 * q:128 * (q + 1)],
                                             a_t[:, 128 * q:128 * (q + 1)])
                        t_ps = prt_ps.tile([128, 128], F16, space="PSUM",
                                           tag="tps")
                        nc.tensor.transpose(out=t_ps[:], in_=h_q[:],
                                            identity=ident[:])
                        hT_q = prt.tile([128, 128], F16, tag="hT")
                        nc.vector.tensor_copy(hT_q[:], t_ps[:])
                        hTs.append(hT_q)
                    return hTs

                # step 1's i/g identity adds run during the Wr load / step 0
                if n_steps > 1:
                    ps_cur = []
                    for c in range(2):
                        gps = pr_ps.tile([128, 512], F32, space="PSUM",
                                         tag=f"gps{c}")
                        ps_cur.append(gps)
                    ident_add(ps_cur[0], xg_tiles[1], 0)
                    ident_add(ps_cur[1], xg_tiles[1], 1)

                # step 0: h = 0 -> gates are just xg
                xg0 = xg_tiles[0]
                act_head(xg0[:, 0:HG], xg0[:, HG:2 * HG])
                hT = act_tail(xg0, 2 * HG, store_out=(n_steps == 1))

                for s in range(1, n_steps):
                    xg_sb = xg_tiles.pop(s)
                    if s + 1 < n_steps:
                        xg_tiles[s + 1] = load_xg(s + 1)
                    ps_b = ps_cur
                    bank_k_mms(ps_b[0], 0, hT)
                    bank_k_mms(ps_b[1], 1, hT)
                    act_head(ps_b[0][:], ps_b[1][:])
                    gps2 = pr_ps.tile([128, 512], F32, space="PSUM",
                                      tag="gps2")
                    ident_add(gps2, xg_sb, 2)
                    bank_k_mms(gps2, 2, hT)
                    if s + 1 < n_steps:
                        # next step's i/g identity adds fill the tail gap
                        ps_cur = []
                        for c in range(2):
                            gps = pr_ps.tile([128, 512], F32, space="PSUM",
                                             tag=f"gps{c}")
                            ps_cur.append(gps)
                        ident_add(ps_cur[0], xg_tiles[s + 1], 0)
                        ident_add(ps_cur[1], xg_tiles[s + 1], 1)
                    hT = act_tail(gps2, 0, store_out=(s == n_steps - 1))
    nc.compile()
    return nc


_BUILT = {}


def _get_built(n_steps=None):
    key = n_steps or N_STEPS
    if key not in _BUILT:
        _BUILT[key] = build(key)
    return _BUILT[key]


def _perm():
    """Row permutation taking PyTorch (i|f|g|o)*2048 rows to 4 groups of
    (i|g|o)*512."""
    p = []
    for j in range(NG):
        for base in (0, 2 * HID, 3 * HID):  # i, g, o blocks
            p.extend(range(base + HG * j, base + HG * (j + 1)))
    return np.array(p)


def prep_inputs(inputs, embed_table, W_ih_f, W_hh_f, b_ih_f, b_hh_f,
                W_ih_b, W_hh_b, b_ih_b, b_hh_b):
    perm = _perm()
    idx = np.asarray(inputs)
    idx = np.where(idx > VOCAB, 0, idx).astype(np.int64)
    idx = np.clip(idx, 0, VOCAB - 1).astype(np.int32)

    Wi_p = np.ascontiguousarray(np.asarray(W_ih_b)[perm].T.astype(np.float16))
    Wf_p = np.ascontiguousarray(np.asarray(W_ih_f)[perm].T.astype(np.float16))
    Wr_p = np.ascontiguousarray(np.asarray(W_hh_b)[perm].T.astype(np.float16))
    bb = (np.asarray(b_ih_b) + np.asarray(b_hh_b))[perm].astype(np.float32)
    bf = (np.asarray(b_ih_f) + np.asarray(b_hh_f))[perm].astype(np.float32)
    bias_b_t = np.ascontiguousarray(np.broadcast_to(bb, (128, G3)))
    # bias_f packed [3, 128, 512]: partition 32j+b -> gate (c, group j)
    bias_f_t = np.empty((3, 128, 512), np.float32)
    for c in range(3):
        for j in range(NG):
            bias_f_t[c, BLOC * j:BLOC * (j + 1), :] = \
                bf[GC * j + 512 * c:GC * j + 512 * (c + 1)]
    table = np.ascontiguousarray(np.asarray(embed_table).astype(np.float16))

    in_maps = []
    for c in range(NCORES):
        sl = idx[BLOC * c:BLOC * (c + 1)]          # [32, 128]
        tok = np.ascontiguousarray(sl[:, ::-1].T.reshape(NTOK, 1))  # t-major rev
        in_maps.append({
            "tok": tok, "table": table, "Wi": Wi_p, "Wf": Wf_p, "Wr": Wr_p,
            "bias_b": bias_b_t, "bias_f": bias_f_t,
        })
    return in_maps


def kernel(**inputs) -> np.ndarray:
    from concourse.bass_utils import run_bass_kernel_spmd
    nc = _get_built()
    in_maps = prep_inputs(**inputs)
    res = run_bass_kernel_spmd(nc, in_maps, core_ids=list(range(NCORES)))
    return np.concatenate([res.results[c]["out"] for c in range(NCORES)], axis=0)


# revision 25
# speedup vs baseline: 1.2561x; 1.1464x over previous
"""Bidirectional-LSTM (degenerate variant) Trainium2 kernel, v3.

Reference semantics: forward direction only uses the last timestep (h/c never
update), backward direction is an h-only recurrence (c stays zero), so only
the i/g/o gates matter:

    h = sig(o) * tanh(sig(i) * tanh(g))
    fwd: gates = x_last @ W_ih_f.T + b_f
    bwd: scan t = S-1..0, gates = x_t @ W_ih_b.T + b_b + h @ W_hh_b.T

Distribution: data-parallel over batch (32 rows/core x 8 cores), weights
replicated.  All matmul operands fp16 (measured end-to-end rel err ~1e-3).

Per core:
  phase 1 (fused): per 128-token m-tile: embedding gather (indirect DMA, fp16
    table) -> one batched XBAR DMA-transpose ([128,1024] -> [128,8,128], off
    the PE) -> input projection with Wi SBUF-resident (two half tiles loaded
    on both HW DMA queues); 4-way M=32 col-tiled matmuls so the four 27ns
    LDWEIGHTS run concurrently instead of one serial 107ns load.  DMAs are
    batched (1 store per gate half) to avoid completion-semaphore convoys.
    Forward cell folded into the last 3 m-tiles (quadrant-packed, Wf
    streamed).
  phase R: 128-step recurrence.  gates = Wr.T @ h, 4-way col-tiled M=32;
    banks i,g first so the sig/tanh chain hides under bank o's matmuls;
    xg+bias folded in as an identity matmul opening each bank's PSUM
    accumulation (no DVE adds; next step's i/g identity matmuls are emitted
    early to fill the inter-step PE gap and keep HAM warm); per-128-col-chunk
    sig(o) -> mul -> PE transpose -> copy pipeline rebuilds the stationary hT
    with K_ORDER consuming chunks in completion order.

Gate columns are host-permuted into 4 groups of (i|g|o) x 512 hid dims so
PSUM column-group j directly yields h[:, 512j:512j+512].
"""

import numpy as np

import concourse.bass as bass
import concourse.bacc as bacc
import concourse.mybir as mybir
import concourse.tile as tile
from concourse.masks import make_identity

VOCAB, EMB, HID = 50000, 1024, 2048
BATCH, SEQ = 256, 128
NCORES = 8
BLOC = BATCH // NCORES            # 32 batch rows per core
NTOK = BLOC * SEQ                 # 4096 tokens per core
NG = 4                            # gate column groups (= hid groups)
GC = 3 * HID // NG                # 1536 gate cols per group (i|g|o x 512)
HG = HID // NG                    # 512 hid dims per group
G3 = 3 * HID                      # 6144 total igo gate cols
MT = NTOK // 128                  # 32 token m-tiles
KT_E = EMB // 128                 # 8 k-tiles, input projection
KT_H = HID // 128                 # 16 k-tiles, recurrence

F32 = mybir.dt.float32
F16 = mybir.dt.float16
I32 = mybir.dt.int32

N_STEPS = SEQ
# k order so hT chunk q (holding k-tiles {q,4+q,8+q,12+q}) is consumed
# chunk-major: the first 4 slots need only chunk 0, etc.
K_ORDER = [0, 4, 8, 12, 1, 5, 9, 13, 2, 6, 10, 14, 3, 7, 11, 15]


def build(n_steps=None):
    n_steps = n_steps or N_STEPS
    nc = bacc.Bacc("TRN2", target_bir_lowering=False, debug=False,
                   num_devices=NCORES)

    tok = nc.dram_tensor("tok", [NTOK, 1], I32, kind="ExternalInput")
    table = nc.dram_tensor("table", [VOCAB, EMB], F16, kind="ExternalInput")
    Wi = nc.dram_tensor("Wi", [EMB, G3], F16, kind="ExternalInput")
    Wf = nc.dram_tensor("Wf", [EMB, G3], F16, kind="ExternalInput")
    Wr = nc.dram_tensor("Wr", [HID, G3], F16, kind="ExternalInput")
    bias_b = nc.dram_tensor("bias_b", [128, G3], F32, kind="ExternalInput")
    # forward bias packed per (c, j): [3, 128, 512], partition 32j+b -> group j
    bias_f = nc.dram_tensor("bias_f", [3, 128, 512], F32, kind="ExternalInput")
    out = nc.dram_tensor("out", [BLOC, 2 * HID], F32, kind="ExternalOutput")

    xgd = nc.dram_tensor("xgd", [NTOK, G3], F16)          # internal

    with tile.TileContext(nc) as tc:
        with tc.tile_pool(name="pk", bufs=1) as pk:
            ident = pk.tile([128, 128], F16)
            make_identity(nc, ident[:])

            # ======== phase 1: gather + transpose + input projection ========
            with tc.tile_pool(name="p1w", bufs=1) as p1w, \
                 tc.tile_pool(name="p1x", bufs=3) as p1x, \
                 tc.tile_pool(name="p1g", bufs=2) as p1g, \
                 tc.tile_pool(name="p1o", bufs=3) as p1o, \
                 tc.tile_pool(name="p1f", bufs=1) as p1f, \
                 tc.tile_pool(name="p1_ps", bufs=1, space="PSUM") as p1_ps, \
                 tc.tile_pool(name="pf_ps", bufs=1, space="PSUM") as pf_ps:
                # Wi halves on both HW DMA queues so bh0 matmuls start early
                wi_h = []
                for bh in range(2):
                    wi_t = p1w.tile([128, KT_E, 3072], F16, tag=f"wi{bh}")
                    eng = nc.sync if bh == 0 else nc.scalar
                    eng.dma_start(
                        out=wi_t[:],
                        in_=Wi[:, 3072 * bh:3072 * (bh + 1)]
                        .rearrange("(k p) c -> p k c", p=128))
                    wi_h.append(wi_t)
                bia_sb = p1w.tile([128, G3], F32, tag="bia")
                nc.sync.dma_start(out=bia_sb[:], in_=bias_b[:, :])
                xt0_sb = p1f.tile([128, KT_E, 128], F16, tag="xt0")

                gf_c = []      # forward gate banks [128, 512] f32
                for m in range(MT):
                    idx_sb = p1g.tile([128, 1], I32, tag="idx")
                    nc.sync.dma_start(out=idx_sb[:],
                                      in_=tok[m * 128:(m + 1) * 128, :])
                    x_sb = p1g.tile([128, EMB], F16, tag="x")
                    nc.gpsimd.indirect_dma_start(
                        out=x_sb[:], out_offset=None, in_=table[:, :],
                        in_offset=bass.IndirectOffsetOnAxis(ap=idx_sb[:, :1], axis=0))
                    # one batched XBAR transpose: xt[p, k, t] = x[t, 128k+p]
                    xt_sb = p1x.tile([128, KT_E, 128], F16, tag="xt")
                    nc.sync.dma_start_transpose(out=xt_sb[:], in_=x_sb[:])
                    if m == 0:
                        # keep m-tile 0 transposed for the forward cell
                        nc.vector.tensor_copy(xt0_sb[:], xt_sb[:])

                    for bh in range(2):
                        xg_sb = p1o.tile([128, 3072], F16, tag="xg")
                        ps_b = []
                        for b in range(6):
                            ps = p1_ps.tile([128, 512], F32, space="PSUM",
                                            tag=f"ps{b}")
                            ps_b.append(ps)
                        for k in range(KT_E):
                            # 4-way M=32 col tiling: four 27ns LDWEIGHTS run
                            # concurrently, vs one serial 107ns full-width
                            for b in range(6):
                                for q in range(NG):
                                    nc.tensor.matmul(
                                        ps_b[b][32 * q:32 * (q + 1), :],
                                        lhsT=xt_sb[:, k, 32 * q:32 * (q + 1)],
                                        rhs=wi_h[bh][:, k, 512 * b:512 * (b + 1)],
                                        start=(k == 0), stop=(k == KT_E - 1),
                                        tile_position=(0, 32 * q),
                                        skip_group_check=True)
                        for b in range(6):
                            cs = 3072 * bh + 512 * b
                            nc.vector.tensor_add(
                                xg_sb[:, 512 * b:512 * (b + 1)], ps_b[b][:],
                                bia_sb[:, cs:cs + 512])
                        nc.scalar.dma_start(
                            out=xgd[m * 128:(m + 1) * 128,
                                    3072 * bh:3072 * (bh + 1)],
                            in_=xg_sb[:])

                    # forward cell: gate bank c at m-tiles 29/30/31
                    if m >= MT - 3:
                        c = m - (MT - 3)
                        wf_js = []
                        for j in range(NG):
                            wf_j = p1f.tile([128, KT_E, 512], F16, tag=f"wf{j}")
                            nc.sync.dma_start(
                                out=wf_j[:],
                                in_=Wf[:, GC * j + 512 * c:GC * j + 512 * (c + 1)]
                                .rearrange("(k p) c -> p k c", p=128))
                            wf_js.append(wf_j)
                        psf = pf_ps.tile([128, 512], F32, space="PSUM",
                                         tag="psf")
                        for k in range(KT_E):
                            lhs = xt0_sb[:, k, 0:BLOC]
                            for j in range(NG):
                                nc.tensor.matmul(
                                    psf[BLOC * j:BLOC * (j + 1), :],
                                    lhsT=lhs, rhs=wf_js[j][:, k, :],
                                    start=(k == 0), stop=(k == KT_E - 1),
                                    tile_position=(0, BLOC * j),
                                    skip_group_check=True)
                        bf_sb = p1f.tile([128, 512], F32, tag=f"bf{c}")
                        nc.sync.dma_start(out=bf_sb[:], in_=bias_f[c, :, :])
                        gf = p1f.tile([128, 512], F32, tag=f"gf{c}")
                        nc.vector.tensor_add(gf[:], psf[:], bf_sb[:])
                        gf_c.append(gf)

                # forward activations: h_f = sig(o)*tanh(sig(i)*tanh(g))
                af = p1f.tile([128, 512], F16, tag="af")
                bf2 = p1f.tile([128, 512], F16, tag="bff")
                nc.scalar.activation(af[:], gf_c[0][:],
                                     mybir.ActivationFunctionType.Sigmoid)
                nc.scalar.activation(bf2[:], gf_c[1][:],
                                     mybir.ActivationFunctionType.Tanh)
                nc.vector.tensor_mul(af[:], af[:], bf2[:])
                nc.scalar.activation(af[:], af[:],
                                     mybir.ActivationFunctionType.Tanh)
                nc.scalar.activation(bf2[:], gf_c[2][:],
                                     mybir.ActivationFunctionType.Sigmoid)
                hf = p1f.tile([128, 512], F32, tag="hf")
                nc.vector.tensor_mul(hf[:], bf2[:], af[:])
                for j in range(NG):
                    nc.sync.dma_start(
                        out=out[:, HG * j:HG * (j + 1)],
                        in_=hf[BLOC * j:BLOC * (j + 1), :])

            tc.strict_bb_all_engine_barrier()
            # ======== phase R: recurrence ========
            with tc.tile_pool(name="prw", bufs=1) as prw, \
                 tc.tile_pool(name="prx", bufs=2) as prx, \
                 tc.tile_pool(name="pra", bufs=1) as pra, \
                 tc.tile_pool(name="prh", bufs=4) as prh, \
                 tc.tile_pool(name="prt", bufs=8) as prt, \
                 tc.tile_pool(name="pr_ps", bufs=1, space="PSUM") as pr_ps, \
                 tc.tile_pool(name="prt_ps", bufs=2, space="PSUM") as prt_ps:
                a_t = pra.tile([128, HG], F16)
                b_t = pra.tile([128, HG], F16)

                def load_xg(s):
                    # partition 32j+b <- xgd[32s+b, 1536j:1536(j+1)]
                    xg_sb = prx.tile([128, GC], F16, tag="xgs")
                    for j in range(NG):
                        nc.sync.dma_start(
                            out=xg_sb[BLOC * j:BLOC * (j + 1), :],
                            in_=xgd[BLOC * s:BLOC * (s + 1),
                                    GC * j:GC * (j + 1)])
                    return xg_sb

                # xg for steps 0/1 first so step 0's chain and step 1's
                # identity adds run while Wr streams in behind them
                xg_tiles = {s: load_xg(s) for s in range(min(2, n_steps))}

                # Wr split: [i|g] cols (needed first) and [o] cols, each
                # loaded as two k-halves on both HW DMA queues
                wr01 = prw.tile([128, KT_H, NG, 1024], F16, tag="wr01")
                wr2 = prw.tile([128, KT_H, NG, 512], F16, tag="wr2")
                wr_v = Wr[:, :].rearrange("(k p) (j c) -> p k j c",
                                          p=128, j=NG)
                kh = KT_H // 2
                for j in range(NG):
                    nc.sync.dma_start(out=wr01[:, 0:kh, j, :],
                                      in_=wr_v[:, 0:kh, j, 0:1024])
                    nc.scalar.dma_start(out=wr01[:, kh:KT_H, j, :],
                                        in_=wr_v[:, kh:KT_H, j, 0:1024])
                for j in range(NG):
                    nc.sync.dma_start(out=wr2[:, 0:kh, j, :],
                                      in_=wr_v[:, 0:kh, j, 1024:1536])
                    nc.scalar.dma_start(out=wr2[:, kh:KT_H, j, :],
                                        in_=wr_v[:, kh:KT_H, j, 1024:1536])

                def ident_add(ps, xg_sb, c):
                    """Open bank c's PSUM accumulation with ps = xg (identity
                    matmul); needs only xg, so next step's i/g adds fill the
                    inter-step PE gap."""
                    nc.tensor.matmul(
                        ps[:], lhsT=ident[:],
                        rhs=xg_sb[:, 512 * c:512 * (c + 1)],
                        start=True, stop=False, skip_group_check=True)

                def bank_slots(ps, c, lhs_of, ki_lo, ki_hi):
                    wr_t = wr01 if c < 2 else wr2
                    co = 512 * c if c < 2 else 0
                    for ki in range(ki_lo, ki_hi):
                        k = K_ORDER[ki]
                        lhs = lhs_of(k)
                        for j in range(NG):
                            nc.tensor.matmul(
                                ps[BLOC * j:BLOC * (j + 1), :],
                                lhsT=lhs,
                                rhs=wr_t[:, k, j, co:co + 512],
                                start=False, stop=(ki == KT_H - 1),
                                tile_position=(0, BLOC * j),
                                skip_group_check=True)

                def bank_k_mms(ps, c, hT):
                    bank_slots(ps, c,
                               lambda k: hT[k % NG][:, BLOC * (k // NG):
                                                    BLOC * (k // NG) + BLOC],
                               0, KT_H)

                def act_head(gi_ap, gg_ap):
                    """a_t = tanh(sig(i) * tanh(g)); runs under bank o's
                    matmuls."""
                    nc.scalar.activation(a_t[:], gi_ap,
                                         mybir.ActivationFunctionType.Sigmoid)
                    nc.scalar.activation(b_t[:], gg_ap,
                                         mybir.ActivationFunctionType.Tanh)
                    nc.vector.tensor_mul(a_t[:], a_t[:], b_t[:])
                    nc.scalar.activation(a_t[:], a_t[:],
                                         mybir.ActivationFunctionType.Tanh)

                def act_tail(go_tile, go_off, store_out, interleave=None):
                    """h = sig(o) * a_t, per-chunk, PE-transposed into 4 hT
                    chunks (kept on the PE: low latency + keeps HAM warm); or
                    the final h stored.  interleave(q, hT_q) emits the next
                    step's bank-0 slot group for chunk q right after its
                    transpose, keeping the PE stream continuous."""
                    if store_out:
                        nc.scalar.activation(
                            b_t[:], go_tile[:, go_off:go_off + HG],
                            mybir.ActivationFunctionType.Sigmoid)
                        h_t = pra.tile([128, HG], F32, tag="hfin")
                        nc.vector.tensor_mul(h_t[:], b_t[:], a_t[:])
                        for j in range(NG):
                            nc.sync.dma_start(
                                out=out[:, HID + HG * j:HID + HG * (j + 1)],
                                in_=h_t[BLOC * j:BLOC * (j + 1), :])
                        return None
                    hTs = []
                    for q in range(NG):
                        nc.scalar.activation(
                            b_t[:, 128 * q:128 * (q + 1)],
                            go_tile[:, go_off + 128 * q:go_off + 128 * (q + 1)],
                            mybir.ActivationFunctionType.Sigmoid)
                        h_q = prh.tile([128, 128], F16, tag="h")
                        nc.vector.tensor_mul(h_q[:],
                                             b_t[:, 128 * q:128 * (q + 1)],
                                             a_t[:, 128 * q:128 * (q + 1)])
                        t_ps = prt_ps.tile([128, 128], F16, space="PSUM",
                                           tag="tps")
                        nc.tensor.transpose(out=t_ps[:], in_=h_q[:],
                                            identity=ident[:])
                        hT_q = prt.tile([128, 128], F16, tag="hT")
                        nc.vector.tensor_copy(hT_q[:], t_ps[:])
                        hTs.append(hT_q)
                        if interleave is not None:
                            interleave(q, hT_q)
                    return hTs

                def alloc_ps01():
                    ps = []
                    for c in range(2):
                        gps = pr_ps.tile([128, 512], F32, space="PSUM",
                                         tag=f"gps{c}")
                        ps.append(gps)
                    return ps

                def b0_interleave(ps0):
                    # bank 0 of the next step, one slot group per hT chunk:
                    # group q's slots K_ORDER[4q:4q+4] all consume chunk q
                    def cb(q, hT_q):
                        bank_slots(ps0, 0,
                                   lambda k: hT_q[:, BLOC * (k // NG):
                                                  BLOC * (k // NG) + BLOC],
                                   4 * q, 4 * (q + 1))
                    return cb

                # step 1's i/g identity adds run during the Wr load / step 0
                if n_steps > 1:
                    ps_cur = alloc_ps01()
                    ident_add(ps_cur[0], xg_tiles[1], 0)
                    ident_add(ps_cur[1], xg_tiles[1], 1)

                # step 0: h = 0 -> gates are just xg
                xg0 = xg_tiles[0]
                act_head(xg0[:, 0:HG], xg0[:, HG:2 * HG])
                hT = act_tail(xg0, 2 * HG, store_out=(n_steps == 1),
                              interleave=(b0_interleave(ps_cur[0])
                                          if n_steps > 1 else None))

                for s in range(1, n_steps):
                    # bank 0 of step s was already emitted, interleaved into
                    # step s-1's tail
                    xg_sb = xg_tiles.pop(s)
                    if s + 1 < n_steps:
                        xg_tiles[s + 1] = load_xg(s + 1)
                    ps_b = ps_cur
                    bank_k_mms(ps_b[1], 1, hT)
                    act_head(ps_b[0][:], ps_b[1][:])
                    gps2 = pr_ps.tile([128, 512], F32, space="PSUM",
                                      tag="gps2")
                    ident_add(gps2, xg_sb, 2)
                    bank_k_mms(gps2, 2, hT)
                    if s + 1 < n_steps:
                        # next step's i/g identity adds fill the tail gap
                        ps_cur = alloc_ps01()
                        ident_add(ps_cur[0], xg_tiles[s + 1], 0)
                        ident_add(ps_cur[1], xg_tiles[s + 1], 1)
                        hT = act_tail(gps2, 0, store_out=False,
                                      interleave=b0_interleave(ps_cur[0]))
                    else:
                        hT = act_tail(gps2, 0, store_out=True)
    nc.compile()
    return nc


_BUILT = {}


def _get_built(n_steps=None):
    key = n_steps or N_STEPS
    if key not in _BUILT:
        _BUILT[key] = build(key)
    return _BUILT[key]


def _perm():
    """Row permutation taking PyTorch (i|f|g|o)*2048 rows to 4 groups of
    (i|g|o)*512."""
    p = []
    for j in range(NG):
        for base in (0, 2 * HID, 3 * HID):  # i, g, o blocks
            p.extend(range(base + HG * j, base + HG * (j + 1)))
    return np.array(p)


def prep_inputs(inputs, embed_table, W_ih_f, W_hh_f, b_ih_f, b_hh_f,
                W_ih_b, W_hh_b, b_ih_b, b_hh_b):
    perm = _perm()
    idx = np.asarray(inputs)
    idx = np.where(idx > VOCAB, 0, idx).astype(np.int64)
    idx = np.clip(idx, 0, VOCAB - 1).astype(np.int32)

    Wi_p = np.ascontiguousarray(np.asarray(W_ih_b)[perm].T.astype(np.float16))
    Wf_p = np.ascontiguousarray(np.asarray(W_ih_f)[perm].T.astype(np.float16))
    Wr_p = np.ascontiguousarray(np.asarray(W_hh_b)[perm].T.astype(np.float16))
    bb = (np.asarray(b_ih_b) + np.asarray(b_hh_b))[perm].astype(np.float32)
    bf = (np.asarray(b_ih_f) + np.asarray(b_hh_f))[perm].astype(np.float32)
    bias_b_t = np.ascontiguousarray(np.broadcast_to(bb, (128, G3)))
    # bias_f packed [3, 128, 512]: partition 32j+b -> gate (c, group j)
    bias_f_t = np.empty((3, 128, 512), np.float32)
    for c in range(3):
        for j in range(NG):
            bias_f_t[c, BLOC * j:BLOC * (j + 1), :] = \
                bf[GC * j + 512 * c:GC * j + 512 * (c + 1)]
    table = np.ascontiguousarray(np.asarray(embed_table).astype(np.float16))

    in_maps = []
    for c in range(NCORES):
        sl = idx[BLOC * c:BLOC * (c + 1)]          # [32, 128]
        tok = np.ascontiguousarray(sl[:, ::-1].T.reshape(NTOK, 1))  # t-major rev
        in_maps.append({
            "tok": tok, "table": table, "Wi": Wi_p, "Wf": Wf_p, "Wr": Wr_p,
            "bias_b": bias_b_t, "bias_f": bias_f_t,
        })
    return in_maps


def kernel(**inputs) -> np.ndarray:
    from concourse.bass_utils import run_bass_kernel_spmd
    nc = _get_built()
    in_maps = prep_inputs(**inputs)
    res = run_bass_kernel_spmd(nc, in_maps, core_ids=list(range(NCORES)))
    return np.concatenate([res.results[c]["out"] for c in range(NCORES)], axis=0)


# revision 27
# speedup vs baseline: 1.2724x; 1.0129x over previous
"""Bidirectional-LSTM (degenerate variant) Trainium2 kernel, v3.

Reference semantics: forward direction only uses the last timestep (h/c never
update), backward direction is an h-only recurrence (c stays zero), so only
the i/g/o gates matter:

    h = sig(o) * tanh(sig(i) * tanh(g))
    fwd: gates = x_last @ W_ih_f.T + b_f
    bwd: scan t = S-1..0, gates = x_t @ W_ih_b.T + b_b + h @ W_hh_b.T

Distribution: data-parallel over batch (32 rows/core x 8 cores), weights
replicated.  All matmul operands fp16 (measured end-to-end rel err ~1e-3).

Per core:
  phase 1 (fused): per 128-token m-tile: embedding gather (indirect DMA, fp16
    table) -> one batched XBAR DMA-transpose ([128,1024] -> [128,8,128], off
    the PE) -> input projection with Wi SBUF-resident (two half tiles loaded
    on both HW DMA queues); 4-way M=32 col-tiled matmuls so the four 27ns
    LDWEIGHTS run concurrently instead of one serial 107ns load.  DMAs are
    batched (1 store per gate half) to avoid completion-semaphore convoys.
    Forward cell folded into the last 3 m-tiles (quadrant-packed, Wf
    streamed).
  phase R: 128-step recurrence.  gates = Wr.T @ h, 4-way col-tiled M=32;
    banks i,g first so the sig/tanh chain hides under bank o's matmuls;
    xg+bias folded in as an identity matmul opening each bank's PSUM
    accumulation (no DVE adds; next step's i/g identity matmuls are emitted
    early to fill the inter-step PE gap and keep HAM warm); per-128-col-chunk
    sig(o) -> mul -> PE transpose -> copy pipeline rebuilds the stationary hT
    with K_ORDER consuming chunks in completion order.

Gate columns are host-permuted into 4 groups of (i|g|o) x 512 hid dims so
PSUM column-group j directly yields h[:, 512j:512j+512].
"""

import numpy as np

import concourse.bass as bass
import concourse.bacc as bacc
import concourse.mybir as mybir
import concourse.tile as tile
from concourse.masks import make_identity

VOCAB, EMB, HID = 50000, 1024, 2048
BATCH, SEQ = 256, 128
NCORES = 8
BLOC = BATCH // NCORES            # 32 batch rows per core
NTOK = BLOC * SEQ                 # 4096 tokens per core
NG = 4                            # gate column groups (= hid groups)
GC = 3 * HID // NG                # 1536 gate cols per group (i|g|o x 512)
HG = HID // NG                    # 512 hid dims per group
G3 = 3 * HID                      # 6144 total igo gate cols
MT = NTOK // 128                  # 32 token m-tiles
KT_E = EMB // 128                 # 8 k-tiles, input projection
KT_H = HID // 128                 # 16 k-tiles, recurrence

F32 = mybir.dt.float32
F16 = mybir.dt.float16
I32 = mybir.dt.int32

N_STEPS = SEQ
# k order so hT chunk q (holding k-tiles {q,4+q,8+q,12+q}) is consumed
# chunk-major: the first 4 slots need only chunk 0, etc.
K_ORDER = [0, 4, 8, 12, 1, 5, 9, 13, 2, 6, 10, 14, 3, 7, 11, 15]


def build(n_steps=None):
    n_steps = n_steps or N_STEPS
    nc = bacc.Bacc("TRN2", target_bir_lowering=False, debug=False,
                   num_devices=NCORES)

    tok = nc.dram_tensor("tok", [NTOK, 1], I32, kind="ExternalInput")
    table = nc.dram_tensor("table", [VOCAB, EMB], F16, kind="ExternalInput")
    Wi = nc.dram_tensor("Wi", [EMB, G3], F16, kind="ExternalInput")
    Wf = nc.dram_tensor("Wf", [EMB, G3], F16, kind="ExternalInput")
    Wr = nc.dram_tensor("Wr", [HID, G3], F16, kind="ExternalInput")
    bias_b = nc.dram_tensor("bias_b", [128, G3], F32, kind="ExternalInput")
    # forward bias packed per (c, j): [3, 128, 512], partition 32j+b -> group j
    bias_f = nc.dram_tensor("bias_f", [3, 128, 512], F32, kind="ExternalInput")
    out = nc.dram_tensor("out", [BLOC, 2 * HID], F32, kind="ExternalOutput")

    xgd = nc.dram_tensor("xgd", [NTOK, G3], F16)          # internal

    with tile.TileContext(nc) as tc:
        with tc.tile_pool(name="pk", bufs=1) as pk:
            ident = pk.tile([128, 128], F16)
            make_identity(nc, ident[:])

            # ======== phase 1: gather + transpose + input projection ========
            with tc.tile_pool(name="p1w", bufs=1) as p1w, \
                 tc.tile_pool(name="p1x", bufs=3) as p1x, \
                 tc.tile_pool(name="p1g", bufs=2) as p1g, \
                 tc.tile_pool(name="p1o", bufs=3) as p1o, \
                 tc.tile_pool(name="p1f", bufs=1) as p1f, \
                 tc.tile_pool(name="p1_ps", bufs=1, space="PSUM") as p1_ps, \
                 tc.tile_pool(name="pf_ps", bufs=1, space="PSUM") as pf_ps:
                # Wi halves on both HW DMA queues so bh0 matmuls start early
                wi_h = []
                for bh in range(2):
                    wi_t = p1w.tile([128, KT_E, 3072], F16, tag=f"wi{bh}")
                    eng = nc.sync if bh == 0 else nc.scalar
                    eng.dma_start(
                        out=wi_t[:],
                        in_=Wi[:, 3072 * bh:3072 * (bh + 1)]
                        .rearrange("(k p) c -> p k c", p=128))
                    wi_h.append(wi_t)
                bia_sb = p1w.tile([128, G3], F32, tag="bia")
                nc.sync.dma_start(out=bia_sb[:], in_=bias_b[:, :])
                xt0_sb = p1f.tile([128, KT_E, 128], F16, tag="xt0")

                gf_c = []      # forward gate banks [128, 512] f32
                psum_rot = 0   # rotate 7 PSUM banks so a half's first
                               # matmul never waits on the previous adds
                for m in range(MT):
                    idx_sb = p1g.tile([128, 1], I32, tag="idx")
                    nc.sync.dma_start(out=idx_sb[:],
                                      in_=tok[m * 128:(m + 1) * 128, :])
                    x_sb = p1g.tile([128, EMB], F16, tag="x")
                    nc.gpsimd.indirect_dma_start(
                        out=x_sb[:], out_offset=None, in_=table[:, :],
                        in_offset=bass.IndirectOffsetOnAxis(ap=idx_sb[:, :1], axis=0))
                    # one batched XBAR transpose: xt[p, k, t] = x[t, 128k+p]
                    xt_sb = p1x.tile([128, KT_E, 128], F16, tag="xt")
                    nc.sync.dma_start_transpose(out=xt_sb[:], in_=x_sb[:])
                    if m == 0:
                        # keep m-tile 0 transposed for the forward cell
                        nc.vector.tensor_copy(xt0_sb[:], xt_sb[:])

                    for bh in range(2):
                        xg_sb = p1o.tile([128, 3072], F16, tag="xg")
                        ps_b = []
                        for b in range(6):
                            ps = p1_ps.tile([128, 512], F32, space="PSUM",
                                            tag=f"ps{(psum_rot + b) % 7}")
                            ps_b.append(ps)
                        psum_rot = (psum_rot + 6) % 7
                        for k in range(KT_E):
                            # 4-way M=32 col tiling: four 27ns LDWEIGHTS run
                            # concurrently, vs one serial 107ns full-width
                            for b in range(6):
                                for q in range(NG):
                                    nc.tensor.matmul(
                                        ps_b[b][32 * q:32 * (q + 1), :],
                                        lhsT=xt_sb[:, k, 32 * q:32 * (q + 1)],
                                        rhs=wi_h[bh][:, k, 512 * b:512 * (b + 1)],
                                        start=(k == 0), stop=(k == KT_E - 1),
                                        tile_position=(0, 32 * q),
                                        skip_group_check=True)
                        for b in range(6):
                            cs = 3072 * bh + 512 * b
                            nc.vector.tensor_add(
                                xg_sb[:, 512 * b:512 * (b + 1)], ps_b[b][:],
                                bia_sb[:, cs:cs + 512])
                        nc.scalar.dma_start(
                            out=xgd[m * 128:(m + 1) * 128,
                                    3072 * bh:3072 * (bh + 1)],
                            in_=xg_sb[:])

                    # forward cell: gate bank c at m-tiles 29/30/31
                    if m >= MT - 3:
                        c = m - (MT - 3)
                        wf_js = []
                        for j in range(NG):
                            wf_j = p1f.tile([128, KT_E, 512], F16, tag=f"wf{j}")
                            nc.sync.dma_start(
                                out=wf_j[:],
                                in_=Wf[:, GC * j + 512 * c:GC * j + 512 * (c + 1)]
                                .rearrange("(k p) c -> p k c", p=128))
                            wf_js.append(wf_j)
                        psf = pf_ps.tile([128, 512], F32, space="PSUM",
                                         tag="psf")
                        for k in range(KT_E):
                            lhs = xt0_sb[:, k, 0:BLOC]
                            for j in range(NG):
                                nc.tensor.matmul(
                                    psf[BLOC * j:BLOC * (j + 1), :],
                                    lhsT=lhs, rhs=wf_js[j][:, k, :],
                                    start=(k == 0), stop=(k == KT_E - 1),
                                    tile_position=(0, BLOC * j),
                                    skip_group_check=True)
                        bf_sb = p1f.tile([128, 512], F32, tag=f"bf{c}")
                        nc.sync.dma_start(out=bf_sb[:], in_=bias_f[c, :, :])
                        gf = p1f.tile([128, 512], F32, tag=f"gf{c}")
                        nc.vector.tensor_add(gf[:], psf[:], bf_sb[:])
                        gf_c.append(gf)

                # forward activations: h_f = sig(o)*tanh(sig(i)*tanh(g))
                af = p1f.tile([128, 512], F16, tag="af")
                bf2 = p1f.tile([128, 512], F16, tag="bff")
                nc.scalar.activation(af[:], gf_c[0][:],
                                     mybir.ActivationFunctionType.Sigmoid)
                nc.scalar.activation(bf2[:], gf_c[1][:],
                                     mybir.ActivationFunctionType.Tanh)
                nc.vector.tensor_mul(af[:], af[:], bf2[:])
                nc.scalar.activation(af[:], af[:],
                                     mybir.ActivationFunctionType.Tanh)
                nc.scalar.activation(bf2[:], gf_c[2][:],
                                     mybir.ActivationFunctionType.Sigmoid)
                hf = p1f.tile([128, 512], F32, tag="hf")
                nc.vector.tensor_mul(hf[:], bf2[:], af[:])
                for j in range(NG):
                    nc.sync.dma_start(
                        out=out[:, HG * j:HG * (j + 1)],
                        in_=hf[BLOC * j:BLOC * (j + 1), :])

            tc.strict_bb_all_engine_barrier()
            # ======== phase R: recurrence ========
            with tc.tile_pool(name="prw", bufs=1) as prw, \
                 tc.tile_pool(name="prx", bufs=2) as prx, \
                 tc.tile_pool(name="pra", bufs=1) as pra, \
                 tc.tile_pool(name="prh", bufs=4) as prh, \
                 tc.tile_pool(name="prt", bufs=8) as prt, \
                 tc.tile_pool(name="pr_ps", bufs=1, space="PSUM") as pr_ps, \
                 tc.tile_pool(name="prt_ps", bufs=2, space="PSUM") as prt_ps:
                a_t = pra.tile([128, HG], F16)
                b_t = pra.tile([128, HG], F16)

                def load_xg(s):
                    # partition 32j+b <- xgd[32s+b, 1536j:1536(j+1)]
                    xg_sb = prx.tile([128, GC], F16, tag="xgs")
                    for j in range(NG):
                        nc.sync.dma_start(
                            out=xg_sb[BLOC * j:BLOC * (j + 1), :],
                            in_=xgd[BLOC * s:BLOC * (s + 1),
                                    GC * j:GC * (j + 1)])
                    return xg_sb

                # xg for steps 0/1 first so step 0's chain and step 1's
                # identity adds run while Wr streams in behind them
                xg_tiles = {s: load_xg(s) for s in range(min(2, n_steps))}

                # Wr split: [i|g] cols (needed first) and [o] cols, each
                # loaded as two k-halves on both HW DMA queues
                wr01 = prw.tile([128, KT_H, NG, 1024], F16, tag="wr01")
                wr2 = prw.tile([128, KT_H, NG, 512], F16, tag="wr2")
                wr_v = Wr[:, :].rearrange("(k p) (j c) -> p k j c",
                                          p=128, j=NG)
                kh = KT_H // 2
                for j in range(NG):
                    nc.sync.dma_start(out=wr01[:, 0:kh, j, :],
                                      in_=wr_v[:, 0:kh, j, 0:1024])
                    nc.scalar.dma_start(out=wr01[:, kh:KT_H, j, :],
                                        in_=wr_v[:, kh:KT_H, j, 0:1024])
                for j in range(NG):
                    nc.sync.dma_start(out=wr2[:, 0:kh, j, :],
                                      in_=wr_v[:, 0:kh, j, 1024:1536])
                    nc.scalar.dma_start(out=wr2[:, kh:KT_H, j, :],
                                        in_=wr_v[:, kh:KT_H, j, 1024:1536])

                def ident_add(ps, xg_sb, c):
                    """Open bank c's PSUM accumulation with ps = xg (identity
                    matmul); needs only xg, so next step's i/g adds fill the
                    inter-step PE gap."""
                    nc.tensor.matmul(
                        ps[:], lhsT=ident[:],
                        rhs=xg_sb[:, 512 * c:512 * (c + 1)],
                        start=True, stop=False, skip_group_check=True)

                def bank_slots(ps, c, lhs_of, ki_lo, ki_hi):
                    wr_t = wr01 if c < 2 else wr2
                    co = 512 * c if c < 2 else 0
                    for ki in range(ki_lo, ki_hi):
                        k = K_ORDER[ki]
                        lhs = lhs_of(k)
                        for j in range(NG):
                            nc.tensor.matmul(
                                ps[BLOC * j:BLOC * (j + 1), :],
                                lhsT=lhs,
                                rhs=wr_t[:, k, j, co:co + 512],
                                start=False, stop=(ki == KT_H - 1),
                                tile_position=(0, BLOC * j),
                                skip_group_check=True)

                def bank_k_mms(ps, c, hT):
                    bank_slots(ps, c,
                               lambda k: hT[k % NG][:, BLOC * (k // NG):
                                                    BLOC * (k // NG) + BLOC],
                               0, KT_H)

                def act_head(gi_ap, gg_ap):
                    """a_t = tanh(sig(i) * tanh(g)); runs under bank o's
                    matmuls."""
                    nc.scalar.activation(a_t[:], gi_ap,
                                         mybir.ActivationFunctionType.Sigmoid)
                    nc.scalar.activation(b_t[:], gg_ap,
                                         mybir.ActivationFunctionType.Tanh)
                    nc.vector.tensor_mul(a_t[:], a_t[:], b_t[:])
                    nc.scalar.activation(a_t[:], a_t[:],
                                         mybir.ActivationFunctionType.Tanh)

                def act_tail(go_tile, go_off, store_out, interleave=None):
                    """h = sig(o) * a_t, per-chunk, PE-transposed into 4 hT
                    chunks (kept on the PE: low latency + keeps HAM warm); or
                    the final h stored.  interleave(q, hT_q) emits the next
                    step's bank-0 slot group for chunk q right after its
                    transpose, keeping the PE stream continuous."""
                    if store_out:
                        nc.scalar.activation(
                            b_t[:], go_tile[:, go_off:go_off + HG],
                            mybir.ActivationFunctionType.Sigmoid)
                        h_t = pra.tile([128, HG], F32, tag="hfin")
                        nc.vector.tensor_mul(h_t[:], b_t[:], a_t[:])
                        for j in range(NG):
                            nc.sync.dma_start(
                                out=out[:, HID + HG * j:HID + HG * (j + 1)],
                                in_=h_t[BLOC * j:BLOC * (j + 1), :])
                        return None
                    hTs = []
                    for q in range(NG):
                        nc.scalar.activation(
                            b_t[:, 128 * q:128 * (q + 1)],
                            go_tile[:, go_off + 128 * q:go_off + 128 * (q + 1)],
                            mybir.ActivationFunctionType.Sigmoid)
                        h_q = prh.tile([128, 128], F16, tag="h")
                        nc.vector.tensor_mul(h_q[:],
                                             b_t[:, 128 * q:128 * (q + 1)],
                                             a_t[:, 128 * q:128 * (q + 1)])
                        t_ps = prt_ps.tile([128, 128], F16, space="PSUM",
                                           tag="tps")
                        nc.tensor.transpose(out=t_ps[:], in_=h_q[:],
                                            identity=ident[:])
                        hT_q = prt.tile([128, 128], F16, tag="hT")
                        nc.vector.tensor_copy(hT_q[:], t_ps[:])
                        hTs.append(hT_q)
                        if interleave is not None:
                            interleave(q, hT_q)
                    return hTs

                def alloc_ps01():
                    ps = []
                    for c in range(2):
                        gps = pr_ps.tile([128, 512], F32, space="PSUM",
                                         tag=f"gps{c}")
                        ps.append(gps)
                    return ps

                def b0_interleave(ps0):
                    # bank 0 of the next step, one slot group per hT chunk:
                    # group q's slots K_ORDER[4q:4q+4] all consume chunk q
                    def cb(q, hT_q):
                        bank_slots(ps0, 0,
                                   lambda k: hT_q[:, BLOC * (k // NG):
                                                  BLOC * (k // NG) + BLOC],
                                   4 * q, 4 * (q + 1))
                    return cb

                # step 1's i/g identity adds run during the Wr load / step 0
                if n_steps > 1:
                    ps_cur = alloc_ps01()
                    ident_add(ps_cur[0], xg_tiles[1], 0)
                    ident_add(ps_cur[1], xg_tiles[1], 1)

                # step 0: h = 0 -> gates are just xg
                xg0 = xg_tiles[0]
                act_head(xg0[:, 0:HG], xg0[:, HG:2 * HG])
                hT = act_tail(xg0, 2 * HG, store_out=(n_steps == 1),
                              interleave=(b0_interleave(ps_cur[0])
                                          if n_steps > 1 else None))

                for s in range(1, n_steps):
                    # bank 0 of step s was already emitted, interleaved into
                    # step s-1's tail
                    xg_sb = xg_tiles.pop(s)
                    if s + 1 < n_steps:
                        xg_tiles[s + 1] = load_xg(s + 1)
                    ps_b = ps_cur
                    bank_k_mms(ps_b[1], 1, hT)
                    act_head(ps_b[0][:], ps_b[1][:])
                    gps2 = pr_ps.tile([128, 512], F32, space="PSUM",
                                      tag="gps2")
                    ident_add(gps2, xg_sb, 2)
                    bank_k_mms(gps2, 2, hT)
                    if s + 1 < n_steps:
                        # next step's i/g identity adds fill the tail gap
                        ps_cur = alloc_ps01()
                        ident_add(ps_cur[0], xg_tiles[s + 1], 0)
                        ident_add(ps_cur[1], xg_tiles[s + 1], 1)
                        hT = act_tail(gps2, 0, store_out=False,
                                      interleave=b0_interleave(ps_cur[0]))
                    else:
                        hT = act_tail(gps2, 0, store_out=True)
    nc.compile()
    return nc


_BUILT = {}


def _get_built(n_steps=None):
    key = n_steps or N_STEPS
    if key not in _BUILT:
        _BUILT[key] = build(key)
    return _BUILT[key]


def _perm():
    """Row permutation taking PyTorch (i|f|g|o)*2048 rows to 4 groups of
    (i|g|o)*512."""
    p = []
    for j in range(NG):
        for base in (0, 2 * HID, 3 * HID):  # i, g, o blocks
            p.extend(range(base + HG * j, base + HG * (j + 1)))
    return np.array(p)


def prep_inputs(inputs, embed_table, W_ih_f, W_hh_f, b_ih_f, b_hh_f,
                W_ih_b, W_hh_b, b_ih_b, b_hh_b):
    perm = _perm()
    idx = np.asarray(inputs)
    idx = np.where(idx > VOCAB, 0, idx).astype(np.int64)
    idx = np.clip(idx, 0, VOCAB - 1).astype(np.int32)

    Wi_p = np.ascontiguousarray(np.asarray(W_ih_b)[perm].T.astype(np.float16))
    Wf_p = np.ascontiguousarray(np.asarray(W_ih_f)[perm].T.astype(np.float16))
    Wr_p = np.ascontiguousarray(np.asarray(W_hh_b)[perm].T.astype(np.float16))
    bb = (np.asarray(b_ih_b) + np.asarray(b_hh_b))[perm].astype(np.float32)
    bf = (np.asarray(b_ih_f) + np.asarray(b_hh_f))[perm].astype(np.float32)
    bias_b_t = np.ascontiguousarray(np.broadcast_to(bb, (128, G3)))
    # bias_f packed [3, 128, 512]: partition 32j+b -> gate (c, group j)
    bias_f_t = np.empty((3, 128, 512), np.float32)
    for c in range(3):
        for j in range(NG):
            bias_f_t[c, BLOC * j:BLOC * (j + 1), :] = \
                bf[GC * j + 512 * c:GC * j + 512 * (c + 1)]
    table = np.ascontiguousarray(np.asarray(embed_table).astype(np.float16))

    in_maps = []
    for c in range(NCORES):
        sl = idx[BLOC * c:BLOC * (c + 1)]          # [32, 128]
        tok = np.ascontiguousarray(sl[:, ::-1].T.reshape(NTOK, 1))  # t-major rev
        in_maps.append({
            "tok": tok, "table": table, "Wi": Wi_p, "Wf": Wf_p, "Wr": Wr_p,
            "bias_b": bias_b_t, "bias_f": bias_f_t,
        })
    return in_maps


def kernel(**inputs) -> np.ndarray:
    from concourse.bass_utils import run_bass_kernel_spmd
    nc = _get_built()
    in_maps = prep_inputs(**inputs)
    res = run_bass_kernel_spmd(nc, in_maps, core_ids=list(range(NCORES)))
    return np.concatenate([res.results[c]["out"] for c in range(NCORES)], axis=0)


# revision 28
# speedup vs baseline: 1.2827x; 1.0082x over previous
"""Bidirectional-LSTM (degenerate variant) Trainium2 kernel, v3.

Reference semantics: forward direction only uses the last timestep (h/c never
update), backward direction is an h-only recurrence (c stays zero), so only
the i/g/o gates matter:

    h = sig(o) * tanh(sig(i) * tanh(g))
    fwd: gates = x_last @ W_ih_f.T + b_f
    bwd: scan t = S-1..0, gates = x_t @ W_ih_b.T + b_b + h @ W_hh_b.T

Distribution: data-parallel over batch (32 rows/core x 8 cores), weights
replicated.  All matmul operands fp16 (measured end-to-end rel err ~1e-3).

Per core:
  phase 1 (fused): per 128-token m-tile: embedding gather (indirect DMA, fp16
    table) -> one batched XBAR DMA-transpose ([128,1024] -> [128,8,128], off
    the PE) -> input projection with Wi SBUF-resident (two half tiles loaded
    on both HW DMA queues); 4-way M=32 col-tiled matmuls so the four 27ns
    LDWEIGHTS run concurrently instead of one serial 107ns load.  DMAs are
    batched (1 store per gate half) to avoid completion-semaphore convoys.
    Forward cell folded into the last 3 m-tiles (quadrant-packed, Wf
    streamed).
  phase R: 128-step recurrence.  gates = Wr.T @ h, 4-way col-tiled M=32;
    banks i,g first so the sig/tanh chain hides under bank o's matmuls;
    xg+bias folded in as an identity matmul opening each bank's PSUM
    accumulation (no DVE adds; next step's i/g identity matmuls are emitted
    early to fill the inter-step PE gap and keep HAM warm); per-128-col-chunk
    sig(o) -> mul -> PE transpose -> copy pipeline rebuilds the stationary hT
    with K_ORDER consuming chunks in completion order.

Gate columns are host-permuted into 4 groups of (i|g|o) x 512 hid dims so
PSUM column-group j directly yields h[:, 512j:512j+512].
"""

import numpy as np

import concourse.bass as bass
import concourse.bacc as bacc
import concourse.mybir as mybir
import concourse.tile as tile
from concourse.masks import make_identity

VOCAB, EMB, HID = 50000, 1024, 2048
BATCH, SEQ = 256, 128
NCORES = 8
BLOC = BATCH // NCORES            # 32 batch rows per core
NTOK = BLOC * SEQ                 # 4096 tokens per core
NG = 4                            # gate column groups (= hid groups)
GC = 3 * HID // NG                # 1536 gate cols per group (i|g|o x 512)
HG = HID // NG                    # 512 hid dims per group
G3 = 3 * HID                      # 6144 total igo gate cols
MT = NTOK // 128                  # 32 token m-tiles
KT_E = EMB // 128                 # 8 k-tiles, input projection
KT_H = HID // 128                 # 16 k-tiles, recurrence

F32 = mybir.dt.float32
F16 = mybir.dt.float16
I32 = mybir.dt.int32

N_STEPS = SEQ
# k order so hT chunk q (holding k-tiles {q,4+q,8+q,12+q}) is consumed
# chunk-major: the first 4 slots need only chunk 0, etc.
K_ORDER = [0, 4, 8, 12, 1, 5, 9, 13, 2, 6, 10, 14, 3, 7, 11, 15]


def build(n_steps=None):
    n_steps = n_steps or N_STEPS
    nc = bacc.Bacc("TRN2", target_bir_lowering=False, debug=False,
                   num_devices=NCORES)

    tok = nc.dram_tensor("tok", [NTOK, 1], I32, kind="ExternalInput")
    table = nc.dram_tensor("table", [VOCAB, EMB], F16, kind="ExternalInput")
    Wi = nc.dram_tensor("Wi", [EMB, G3], F16, kind="ExternalInput")
    Wf = nc.dram_tensor("Wf", [EMB, G3], F16, kind="ExternalInput")
    Wr = nc.dram_tensor("Wr", [HID, G3], F16, kind="ExternalInput")
    bias_b = nc.dram_tensor("bias_b", [128, G3], F32, kind="ExternalInput")
    # forward bias packed per (c, j): [3, 128, 512], partition 32j+b -> group j
    bias_f = nc.dram_tensor("bias_f", [3, 128, 512], F32, kind="ExternalInput")
    out = nc.dram_tensor("out", [BLOC, 2 * HID], F32, kind="ExternalOutput")

    xgd = nc.dram_tensor("xgd", [NTOK, G3], F16)          # internal

    with tile.TileContext(nc) as tc:
        with tc.tile_pool(name="pk", bufs=1) as pk:
            ident = pk.tile([128, 128], F16)
            make_identity(nc, ident[:])

            # ======== phase 1: gather + transpose + input projection ========
            with tc.tile_pool(name="p1w", bufs=1) as p1w, \
                 tc.tile_pool(name="p1x", bufs=3) as p1x, \
                 tc.tile_pool(name="p1g", bufs=2) as p1g, \
                 tc.tile_pool(name="p1o", bufs=3) as p1o, \
                 tc.tile_pool(name="p1f", bufs=1) as p1f, \
                 tc.tile_pool(name="p1_ps", bufs=1, space="PSUM") as p1_ps, \
                 tc.tile_pool(name="pf_ps", bufs=1, space="PSUM") as pf_ps:
                def emit_gather(m):
                    idx_sb = p1g.tile([128, 1], I32, tag="idx")
                    nc.sync.dma_start(out=idx_sb[:],
                                      in_=tok[m * 128:(m + 1) * 128, :])
                    x_sb = p1g.tile([128, EMB], F16, tag="x")
                    nc.gpsimd.indirect_dma_start(
                        out=x_sb[:], out_offset=None, in_=table[:, :],
                        in_offset=bass.IndirectOffsetOnAxis(ap=idx_sb[:, :1], axis=0))
                    # one batched XBAR transpose: xt[p, k, t] = x[t, 128k+p]
                    xt_sb = p1x.tile([128, KT_E, 128], F16, tag="xt")
                    nc.sync.dma_start_transpose(out=xt_sb[:], in_=x_sb[:])
                    return xt_sb

                # m-tile 0's gather goes ahead of the weight loads on the
                # sync queue so its transpose is ready when Wi lands
                xt_first = emit_gather(0)
                # Wi halves on both HW DMA queues so bh0 matmuls start early
                wi_h = []
                for bh in range(2):
                    wi_t = p1w.tile([128, KT_E, 3072], F16, tag=f"wi{bh}")
                    eng = nc.sync if bh == 0 else nc.scalar
                    eng.dma_start(
                        out=wi_t[:],
                        in_=Wi[:, 3072 * bh:3072 * (bh + 1)]
                        .rearrange("(k p) c -> p k c", p=128))
                    wi_h.append(wi_t)
                bia_sb = p1w.tile([128, G3], F32, tag="bia")
                # bias rides the scalar queue; not needed until the first adds
                nc.scalar.dma_start(out=bia_sb[:], in_=bias_b[:, :])
                xt0_sb = p1f.tile([128, KT_E, 128], F16, tag="xt0")

                gf_c = []      # forward gate banks [128, 512] f32
                psum_rot = 0   # rotate 7 PSUM banks so a half's first
                               # matmul never waits on the previous adds
                for m in range(MT):
                    xt_sb = xt_first if m == 0 else emit_gather(m)
                    if m == 0:
                        # keep m-tile 0 transposed for the forward cell
                        nc.vector.tensor_copy(xt0_sb[:], xt_sb[:])

                    for bh in range(2):
                        xg_sb = p1o.tile([128, 3072], F16, tag="xg")
                        ps_b = []
                        for b in range(6):
                            ps = p1_ps.tile([128, 512], F32, space="PSUM",
                                            tag=f"ps{(psum_rot + b) % 7}")
                            ps_b.append(ps)
                        psum_rot = (psum_rot + 6) % 7
                        for k in range(KT_E):
                            # 4-way M=32 col tiling: four 27ns LDWEIGHTS run
                            # concurrently, vs one serial 107ns full-width
                            for b in range(6):
                                for q in range(NG):
                                    nc.tensor.matmul(
                                        ps_b[b][32 * q:32 * (q + 1), :],
                                        lhsT=xt_sb[:, k, 32 * q:32 * (q + 1)],
                                        rhs=wi_h[bh][:, k, 512 * b:512 * (b + 1)],
                                        start=(k == 0), stop=(k == KT_E - 1),
                                        tile_position=(0, 32 * q),
                                        skip_group_check=True)
                        for b in range(6):
                            cs = 3072 * bh + 512 * b
                            nc.vector.tensor_add(
                                xg_sb[:, 512 * b:512 * (b + 1)], ps_b[b][:],
                                bia_sb[:, cs:cs + 512])
                        nc.scalar.dma_start(
                            out=xgd[m * 128:(m + 1) * 128,
                                    3072 * bh:3072 * (bh + 1)],
                            in_=xg_sb[:])

                    # forward cell: gate bank c at m-tiles 29/30/31
                    if m >= MT - 3:
                        c = m - (MT - 3)
                        wf_js = []
                        for j in range(NG):
                            wf_j = p1f.tile([128, KT_E, 512], F16, tag=f"wf{j}")
                            nc.sync.dma_start(
                                out=wf_j[:],
                                in_=Wf[:, GC * j + 512 * c:GC * j + 512 * (c + 1)]
                                .rearrange("(k p) c -> p k c", p=128))
                            wf_js.append(wf_j)
                        psf = pf_ps.tile([128, 512], F32, space="PSUM",
                                         tag="psf")
                        for k in range(KT_E):
                            lhs = xt0_sb[:, k, 0:BLOC]
                            for j in range(NG):
                                nc.tensor.matmul(
                                    psf[BLOC * j:BLOC * (j + 1), :],
                                    lhsT=lhs, rhs=wf_js[j][:, k, :],
                                    start=(k == 0), stop=(k == KT_E - 1),
                                    tile_position=(0, BLOC * j),
                                    skip_group_check=True)
                        bf_sb = p1f.tile([128, 512], F32, tag=f"bf{c}")
                        nc.sync.dma_start(out=bf_sb[:], in_=bias_f[c, :, :])
                        gf = p1f.tile([128, 512], F32, tag=f"gf{c}")
                        nc.vector.tensor_add(gf[:], psf[:], bf_sb[:])
                        gf_c.append(gf)

                # forward activations: h_f = sig(o)*tanh(sig(i)*tanh(g))
                af = p1f.tile([128, 512], F16, tag="af")
                bf2 = p1f.tile([128, 512], F16, tag="bff")
                nc.scalar.activation(af[:], gf_c[0][:],
                                     mybir.ActivationFunctionType.Sigmoid)
                nc.scalar.activation(bf2[:], gf_c[1][:],
                                     mybir.ActivationFunctionType.Tanh)
                nc.vector.tensor_mul(af[:], af[:], bf2[:])
                nc.scalar.activation(af[:], af[:],
                                     mybir.ActivationFunctionType.Tanh)
                nc.scalar.activation(bf2[:], gf_c[2][:],
                                     mybir.ActivationFunctionType.Sigmoid)
                hf = p1f.tile([128, 512], F32, tag="hf")
                nc.vector.tensor_mul(hf[:], bf2[:], af[:])
                for j in range(NG):
                    nc.sync.dma_start(
                        out=out[:, HG * j:HG * (j + 1)],
                        in_=hf[BLOC * j:BLOC * (j + 1), :])

            tc.strict_bb_all_engine_barrier()
            # ======== phase R: recurrence ========
            with tc.tile_pool(name="prw", bufs=1) as prw, \
                 tc.tile_pool(name="prx", bufs=2) as prx, \
                 tc.tile_pool(name="pra", bufs=1) as pra, \
                 tc.tile_pool(name="prh", bufs=4) as prh, \
                 tc.tile_pool(name="prt", bufs=8) as prt, \
                 tc.tile_pool(name="pr_ps", bufs=1, space="PSUM") as pr_ps, \
                 tc.tile_pool(name="prt_ps", bufs=2, space="PSUM") as prt_ps:
                a_t = pra.tile([128, HG], F16)
                b_t = pra.tile([128, HG], F16)

                def load_xg(s):
                    # partition 32j+b <- xgd[32s+b, 1536j:1536(j+1)]
                    xg_sb = prx.tile([128, GC], F16, tag="xgs")
                    for j in range(NG):
                        nc.sync.dma_start(
                            out=xg_sb[BLOC * j:BLOC * (j + 1), :],
                            in_=xgd[BLOC * s:BLOC * (s + 1),
                                    GC * j:GC * (j + 1)])
                    return xg_sb

                # xg for steps 0/1 first so step 0's chain and step 1's
                # identity adds run while Wr streams in behind them
                xg_tiles = {s: load_xg(s) for s in range(min(2, n_steps))}

                # Wr split: [i|g] cols (needed first) and [o] cols, each
                # loaded as two k-halves on both HW DMA queues
                wr01 = prw.tile([128, KT_H, NG, 1024], F16, tag="wr01")
                wr2 = prw.tile([128, KT_H, NG, 512], F16, tag="wr2")
                wr_v = Wr[:, :].rearrange("(k p) (j c) -> p k j c",
                                          p=128, j=NG)
                kh = KT_H // 2
                for j in range(NG):
                    nc.sync.dma_start(out=wr01[:, 0:kh, j, :],
                                      in_=wr_v[:, 0:kh, j, 0:1024])
                    nc.scalar.dma_start(out=wr01[:, kh:KT_H, j, :],
                                        in_=wr_v[:, kh:KT_H, j, 0:1024])
                for j in range(NG):
                    nc.sync.dma_start(out=wr2[:, 0:kh, j, :],
                                      in_=wr_v[:, 0:kh, j, 1024:1536])
                    nc.scalar.dma_start(out=wr2[:, kh:KT_H, j, :],
                                        in_=wr_v[:, kh:KT_H, j, 1024:1536])

                def ident_add(ps, xg_sb, c):
                    """Open bank c's PSUM accumulation with ps = xg (identity
                    matmul); needs only xg, so next step's i/g adds fill the
                    inter-step PE gap."""
                    nc.tensor.matmul(
                        ps[:], lhsT=ident[:],
                        rhs=xg_sb[:, 512 * c:512 * (c + 1)],
                        start=True, stop=False, skip_group_check=True)

                def bank_slots(ps, c, lhs_of, ki_lo, ki_hi):
                    wr_t = wr01 if c < 2 else wr2
                    co = 512 * c if c < 2 else 0
                    for ki in range(ki_lo, ki_hi):
                        k = K_ORDER[ki]
                        lhs = lhs_of(k)
                        for j in range(NG):
                            nc.tensor.matmul(
                                ps[BLOC * j:BLOC * (j + 1), :],
                                lhsT=lhs,
                                rhs=wr_t[:, k, j, co:co + 512],
                                start=False, stop=(ki == KT_H - 1),
                                tile_position=(0, BLOC * j),
                                skip_group_check=True)

                def bank_k_mms(ps, c, hT):
                    bank_slots(ps, c,
                               lambda k: hT[k % NG][:, BLOC * (k // NG):
                                                    BLOC * (k // NG) + BLOC],
                               0, KT_H)

                def act_head(gi_ap, gg_ap):
                    """a_t = tanh(sig(i) * tanh(g)); runs under bank o's
                    matmuls."""
                    nc.scalar.activation(a_t[:], gi_ap,
                                         mybir.ActivationFunctionType.Sigmoid)
                    nc.scalar.activation(b_t[:], gg_ap,
                                         mybir.ActivationFunctionType.Tanh)
                    nc.vector.tensor_mul(a_t[:], a_t[:], b_t[:])
                    nc.scalar.activation(a_t[:], a_t[:],
                                         mybir.ActivationFunctionType.Tanh)

                def act_tail(go_tile, go_off, store_out, interleave=None):
                    """h = sig(o) * a_t, per-chunk, PE-transposed into 4 hT
                    chunks (kept on the PE: low latency + keeps HAM warm); or
                    the final h stored.  interleave(q, hT_q) emits the next
                    step's bank-0 slot group for chunk q right after its
                    transpose, keeping the PE stream continuous."""
                    if store_out:
                        nc.scalar.activation(
                            b_t[:], go_tile[:, go_off:go_off + HG],
                            mybir.ActivationFunctionType.Sigmoid)
                        h_t = pra.tile([128, HG], F32, tag="hfin")
                        nc.vector.tensor_mul(h_t[:], b_t[:], a_t[:])
                        for j in range(NG):
                            nc.sync.dma_start(
                                out=out[:, HID + HG * j:HID + HG * (j + 1)],
                                in_=h_t[BLOC * j:BLOC * (j + 1), :])
                        return None
                    hTs = []
                    for q in range(NG):
                        nc.scalar.activation(
                            b_t[:, 128 * q:128 * (q + 1)],
                            go_tile[:, go_off + 128 * q:go_off + 128 * (q + 1)],
                            mybir.ActivationFunctionType.Sigmoid)
                        h_q = prh.tile([128, 128], F16, tag="h")
                        nc.vector.tensor_mul(h_q[:],
                                             b_t[:, 128 * q:128 * (q + 1)],
                                             a_t[:, 128 * q:128 * (q + 1)])
                        t_ps = prt_ps.tile([128, 128], F16, space="PSUM",
                                           tag="tps")
                        nc.tensor.transpose(out=t_ps[:], in_=h_q[:],
                                            identity=ident[:])
                        hT_q = prt.tile([128, 128], F16, tag="hT")
                        nc.vector.tensor_copy(hT_q[:], t_ps[:])
                        hTs.append(hT_q)
                        if interleave is not None:
                            interleave(q, hT_q)
                    return hTs

                def alloc_ps01():
                    ps = []
                    for c in range(2):
                        gps = pr_ps.tile([128, 512], F32, space="PSUM",
                                         tag=f"gps{c}")
                        ps.append(gps)
                    return ps

                def b0_interleave(ps0):
                    # bank 0 of the next step, one slot group per hT chunk:
                    # group q's slots K_ORDER[4q:4q+4] all consume chunk q
                    def cb(q, hT_q):
                        bank_slots(ps0, 0,
                                   lambda k: hT_q[:, BLOC * (k // NG):
                                                  BLOC * (k // NG) + BLOC],
                                   4 * q, 4 * (q + 1))
                    return cb

                # step 1's i/g identity adds run during the Wr load / step 0
                if n_steps > 1:
                    ps_cur = alloc_ps01()
                    ident_add(ps_cur[0], xg_tiles[1], 0)
                    ident_add(ps_cur[1], xg_tiles[1], 1)

                # step 0: h = 0 -> gates are just xg
                xg0 = xg_tiles[0]
                act_head(xg0[:, 0:HG], xg0[:, HG:2 * HG])
                hT = act_tail(xg0, 2 * HG, store_out=(n_steps == 1),
                              interleave=(b0_interleave(ps_cur[0])
                                          if n_steps > 1 else None))

                for s in range(1, n_steps):
                    # bank 0 of step s was already emitted, interleaved into
                    # step s-1's tail
                    xg_sb = xg_tiles.pop(s)
                    if s + 1 < n_steps:
                        xg_tiles[s + 1] = load_xg(s + 1)
                    ps_b = ps_cur
                    bank_k_mms(ps_b[1], 1, hT)
                    act_head(ps_b[0][:], ps_b[1][:])
                    gps2 = pr_ps.tile([128, 512], F32, space="PSUM",
                                      tag="gps2")
                    ident_add(gps2, xg_sb, 2)
                    bank_k_mms(gps2, 2, hT)
                    if s + 1 < n_steps:
                        # next step's i/g identity adds fill the tail gap
                        ps_cur = alloc_ps01()
                        ident_add(ps_cur[0], xg_tiles[s + 1], 0)
                        ident_add(ps_cur[1], xg_tiles[s + 1], 1)
                        hT = act_tail(gps2, 0, store_out=False,
                                      interleave=b0_interleave(ps_cur[0]))
                    else:
                        hT = act_tail(gps2, 0, store_out=True)
    nc.compile()
    return nc


_BUILT = {}


def _get_built(n_steps=None):
    key = n_steps or N_STEPS
    if key not in _BUILT:
        _BUILT[key] = build(key)
    return _BUILT[key]


def _perm():
    """Row permutation taking PyTorch (i|f|g|o)*2048 rows to 4 groups of
    (i|g|o)*512."""
    p = []
    for j in range(NG):
        for base in (0, 2 * HID, 3 * HID):  # i, g, o blocks
            p.extend(range(base + HG * j, base + HG * (j + 1)))
    return np.array(p)


def prep_inputs(inputs, embed_table, W_ih_f, W_hh_f, b_ih_f, b_hh_f,
                W_ih_b, W_hh_b, b_ih_b, b_hh_b):
    perm = _perm()
    idx = np.asarray(inputs)
    idx = np.where(idx > VOCAB, 0, idx).astype(np.int64)
    idx = np.clip(idx, 0, VOCAB - 1).astype(np.int32)

    Wi_p = np.ascontiguousarray(np.asarray(W_ih_b)[perm].T.astype(np.float16))
    Wf_p = np.ascontiguousarray(np.asarray(W_ih_f)[perm].T.astype(np.float16))
    Wr_p = np.ascontiguousarray(np.asarray(W_hh_b)[perm].T.astype(np.float16))
    bb = (np.asarray(b_ih_b) + np.asarray(b_hh_b))[perm].astype(np.float32)
    bf = (np.asarray(b_ih_f) + np.asarray(b_hh_f))[perm].astype(np.float32)
    bias_b_t = np.ascontiguousarray(np.broadcast_to(bb, (128, G3)))
    # bias_f packed [3, 128, 512]: partition 32j+b -> gate (c, group j)
    bias_f_t = np.empty((3, 128, 512), np.float32)
    for c in range(3):
        for j in range(NG):
            bias_f_t[c, BLOC * j:BLOC * (j + 1), :] = \
                bf[GC * j + 512 * c:GC * j + 512 * (c + 1)]
    table = np.ascontiguousarray(np.asarray(embed_table).astype(np.float16))

    in_maps = []
    for c in range(NCORES):
        sl = idx[BLOC * c:BLOC * (c + 1)]          # [32, 128]
        tok = np.ascontiguousarray(sl[:, ::-1].T.reshape(NTOK, 1))  # t-major rev
        in_maps.append({
            "tok": tok, "table": table, "Wi": Wi_p, "Wf": Wf_p, "Wr": Wr_p,
            "bias_b": bias_b_t, "bias_f": bias_f_t,
        })
    return in_maps


def kernel(**inputs) -> np.ndarray:
    from concourse.bass_utils import run_bass_kernel_spmd
    nc = _get_built()
    in_maps = prep_inputs(**inputs)
    res = run_bass_kernel_spmd(nc, in_maps, core_ids=list(range(NCORES)))
    return np.concatenate([res.results[c]["out"] for c in range(NCORES)], axis=0)


# revision 29
# speedup vs baseline: 1.2872x; 1.0035x over previous
"""Bidirectional-LSTM (degenerate variant) Trainium2 kernel, v3.

Reference semantics: forward direction only uses the last timestep (h/c never
update), backward direction is an h-only recurrence (c stays zero), so only
the i/g/o gates matter:

    h = sig(o) * tanh(sig(i) * tanh(g))
    fwd: gates = x_last @ W_ih_f.T + b_f
    bwd: scan t = S-1..0, gates = x_t @ W_ih_b.T + b_b + h @ W_hh_b.T

Distribution: data-parallel over batch (32 rows/core x 8 cores), weights
replicated.  All matmul operands fp16 (measured end-to-end rel err ~1e-3).

Per core:
  phase 1 (fused): per 128-token m-tile: embedding gather (indirect DMA, fp16
    table) -> one batched XBAR DMA-transpose ([128,1024] -> [128,8,128], off
    the PE) -> input projection with Wi SBUF-resident (two half tiles loaded
    on both HW DMA queues); 4-way M=32 col-tiled matmuls so the four 27ns
    LDWEIGHTS run concurrently instead of one serial 107ns load.  DMAs are
    batched (1 store per gate half) to avoid completion-semaphore convoys.
    Forward cell folded into the last 3 m-tiles (quadrant-packed, Wf
    streamed).
  phase R: 128-step recurrence.  gates = Wr.T @ h, 4-way col-tiled M=32;
    banks i,g first so the sig/tanh chain hides under bank o's matmuls;
    xg+bias folded in as an identity matmul opening each bank's PSUM
    accumulation (no DVE adds; next step's i/g identity matmuls are emitted
    early to fill the inter-step PE gap and keep HAM warm); per-128-col-chunk
    sig(o) -> mul -> PE transpose -> copy pipeline rebuilds the stationary hT
    with K_ORDER consuming chunks in completion order.

Gate columns are host-permuted into 4 groups of (i|g|o) x 512 hid dims so
PSUM column-group j directly yields h[:, 512j:512j+512].
"""

import numpy as np

import concourse.bass as bass
import concourse.bacc as bacc
import concourse.mybir as mybir
import concourse.tile as tile
from concourse.masks import make_identity

VOCAB, EMB, HID = 50000, 1024, 2048
BATCH, SEQ = 256, 128
NCORES = 8
BLOC = BATCH // NCORES            # 32 batch rows per core
NTOK = BLOC * SEQ                 # 4096 tokens per core
NG = 4                            # gate column groups (= hid groups)
GC = 3 * HID // NG                # 1536 gate cols per group (i|g|o x 512)
HG = HID // NG                    # 512 hid dims per group
G3 = 3 * HID                      # 6144 total igo gate cols
MT = NTOK // 128                  # 32 token m-tiles
KT_E = EMB // 128                 # 8 k-tiles, input projection
KT_H = HID // 128                 # 16 k-tiles, recurrence

F32 = mybir.dt.float32
F16 = mybir.dt.float16
I32 = mybir.dt.int32

N_STEPS = SEQ
# k order so hT chunk q (holding k-tiles {q,4+q,8+q,12+q}) is consumed
# chunk-major: the first 4 slots need only chunk 0, etc.
K_ORDER = [0, 4, 8, 12, 1, 5, 9, 13, 2, 6, 10, 14, 3, 7, 11, 15]


def build(n_steps=None):
    n_steps = n_steps or N_STEPS
    nc = bacc.Bacc("TRN2", target_bir_lowering=False, debug=False,
                   num_devices=NCORES)

    tok = nc.dram_tensor("tok", [NTOK, 1], I32, kind="ExternalInput")
    table = nc.dram_tensor("table", [VOCAB, EMB], F16, kind="ExternalInput")
    Wi = nc.dram_tensor("Wi", [EMB, G3], F16, kind="ExternalInput")
    Wf = nc.dram_tensor("Wf", [EMB, G3], F16, kind="ExternalInput")
    Wr = nc.dram_tensor("Wr", [HID, G3], F16, kind="ExternalInput")
    bias_b = nc.dram_tensor("bias_b", [128, G3], F32, kind="ExternalInput")
    # forward bias packed per (c, j): [3, 128, 512], partition 32j+b -> group j
    bias_f = nc.dram_tensor("bias_f", [3, 128, 512], F32, kind="ExternalInput")
    out = nc.dram_tensor("out", [BLOC, 2 * HID], F32, kind="ExternalOutput")

    xgd = nc.dram_tensor("xgd", [NTOK, G3], F16)          # internal

    with tile.TileContext(nc) as tc:
        with tc.tile_pool(name="pk", bufs=1) as pk:
            ident = pk.tile([128, 128], F16)
            make_identity(nc, ident[:])

            # ======== phase 1: gather + transpose + input projection ========
            with tc.tile_pool(name="p1w", bufs=1) as p1w, \
                 tc.tile_pool(name="p1x", bufs=3) as p1x, \
                 tc.tile_pool(name="p1g", bufs=2) as p1g, \
                 tc.tile_pool(name="p1o", bufs=3) as p1o, \
                 tc.tile_pool(name="p1f", bufs=1) as p1f, \
                 tc.tile_pool(name="p1_ps", bufs=1, space="PSUM") as p1_ps, \
                 tc.tile_pool(name="pf_ps", bufs=1, space="PSUM") as pf_ps:
                def emit_gather(m):
                    idx_sb = p1g.tile([128, 1], I32, tag="idx")
                    nc.sync.dma_start(out=idx_sb[:],
                                      in_=tok[m * 128:(m + 1) * 128, :])
                    x_sb = p1g.tile([128, EMB], F16, tag="x")
                    nc.gpsimd.indirect_dma_start(
                        out=x_sb[:], out_offset=None, in_=table[:, :],
                        in_offset=bass.IndirectOffsetOnAxis(ap=idx_sb[:, :1], axis=0))
                    # one batched XBAR transpose: xt[p, k, t] = x[t, 128k+p]
                    xt_sb = p1x.tile([128, KT_E, 128], F16, tag="xt")
                    nc.sync.dma_start_transpose(out=xt_sb[:], in_=x_sb[:])
                    return xt_sb

                # m-tile 0's gather goes ahead of the weight loads on the
                # sync queue so its transpose is ready when Wi lands
                xt_first = emit_gather(0)
                # Wi halves on both HW DMA queues so bh0 matmuls start early
                wi_h = []
                for bh in range(2):
                    wi_t = p1w.tile([128, KT_E, 3072], F16, tag=f"wi{bh}")
                    eng = nc.sync if bh == 0 else nc.scalar
                    eng.dma_start(
                        out=wi_t[:],
                        in_=Wi[:, 3072 * bh:3072 * (bh + 1)]
                        .rearrange("(k p) c -> p k c", p=128))
                    wi_h.append(wi_t)
                bia_sb = p1w.tile([128, G3], F32, tag="bia")
                # bias rides the scalar queue; not needed until the first adds
                nc.scalar.dma_start(out=bia_sb[:], in_=bias_b[:, :])
                xt0_sb = p1f.tile([128, KT_E, 128], F16, tag="xt0")

                gf_c = []      # forward gate banks [128, 512] f32
                psum_rot = 0   # rotate 7 PSUM banks so a half's first
                               # matmul never waits on the previous adds
                for m in range(MT):
                    xt_sb = xt_first if m == 0 else emit_gather(m)
                    if m == 0:
                        # keep m-tile 0 transposed for the forward cell
                        nc.vector.tensor_copy(xt0_sb[:], xt_sb[:])

                    for bh in range(2):
                        xg_sb = p1o.tile([128, 3072], F16, tag="xg")
                        ps_b = []
                        for b in range(6):
                            ps = p1_ps.tile([128, 512], F32, space="PSUM",
                                            tag=f"ps{(psum_rot + b) % 7}")
                            ps_b.append(ps)
                        psum_rot = (psum_rot + 6) % 7
                        for k in range(KT_E):
                            # 4-way M=32 col tiling: four 27ns LDWEIGHTS run
                            # concurrently, vs one serial 107ns full-width
                            for b in range(6):
                                for q in range(NG):
                                    nc.tensor.matmul(
                                        ps_b[b][32 * q:32 * (q + 1), :],
                                        lhsT=xt_sb[:, k, 32 * q:32 * (q + 1)],
                                        rhs=wi_h[bh][:, k, 512 * b:512 * (b + 1)],
                                        start=(k == 0), stop=(k == KT_E - 1),
                                        tile_position=(0, 32 * q),
                                        skip_group_check=True)
                        for b in range(6):
                            cs = 3072 * bh + 512 * b
                            nc.vector.tensor_add(
                                xg_sb[:, 512 * b:512 * (b + 1)], ps_b[b][:],
                                bia_sb[:, cs:cs + 512])
                        nc.scalar.dma_start(
                            out=xgd[m * 128:(m + 1) * 128,
                                    3072 * bh:3072 * (bh + 1)],
                            in_=xg_sb[:])

                    # forward cell: gate bank c at m-tiles 29/30/31
                    if m >= MT - 3:
                        c = m - (MT - 3)
                        wf_js = []
                        for j in range(NG):
                            wf_j = p1f.tile([128, KT_E, 512], F16, tag=f"wf{j}")
                            nc.sync.dma_start(
                                out=wf_j[:],
                                in_=Wf[:, GC * j + 512 * c:GC * j + 512 * (c + 1)]
                                .rearrange("(k p) c -> p k c", p=128))
                            wf_js.append(wf_j)
                        psf = pf_ps.tile([128, 512], F32, space="PSUM",
                                         tag="psf")
                        for k in range(KT_E):
                            lhs = xt0_sb[:, k, 0:BLOC]
                            for j in range(NG):
                                nc.tensor.matmul(
                                    psf[BLOC * j:BLOC * (j + 1), :],
                                    lhsT=lhs, rhs=wf_js[j][:, k, :],
                                    start=(k == 0), stop=(k == KT_E - 1),
                                    tile_position=(0, BLOC * j),
                                    skip_group_check=True)
                        bf_sb = p1f.tile([128, 512], F32, tag=f"bf{c}")
                        nc.sync.dma_start(out=bf_sb[:], in_=bias_f[c, :, :])
                        gf = p1f.tile([128, 512], F32, tag=f"gf{c}")
                        nc.vector.tensor_add(gf[:], psf[:], bf_sb[:])
                        gf_c.append(gf)

                # forward activations: h_f = sig(o)*tanh(sig(i)*tanh(g))
                af = p1f.tile([128, 512], F16, tag="af")
                bf2 = p1f.tile([128, 512], F16, tag="bff")
                nc.scalar.activation(af[:], gf_c[0][:],
                                     mybir.ActivationFunctionType.Sigmoid)
                nc.scalar.activation(bf2[:], gf_c[1][:],
                                     mybir.ActivationFunctionType.Tanh)
                nc.vector.tensor_mul(af[:], af[:], bf2[:])
                nc.scalar.activation(af[:], af[:],
                                     mybir.ActivationFunctionType.Tanh)
                nc.scalar.activation(bf2[:], gf_c[2][:],
                                     mybir.ActivationFunctionType.Sigmoid)
                hf = p1f.tile([128, 512], F32, tag="hf")
                nc.vector.tensor_mul(hf[:], bf2[:], af[:])
                for j in range(NG):
                    nc.sync.dma_start(
                        out=out[:, HG * j:HG * (j + 1)],
                        in_=hf[BLOC * j:BLOC * (j + 1), :])

            tc.strict_bb_all_engine_barrier()
            # ======== phase R: recurrence ========
            with tc.tile_pool(name="prw", bufs=1) as prw, \
                 tc.tile_pool(name="prx", bufs=2) as prx, \
                 tc.tile_pool(name="pra", bufs=1) as pra, \
                 tc.tile_pool(name="prh", bufs=4) as prh, \
                 tc.tile_pool(name="prt", bufs=8) as prt, \
                 tc.tile_pool(name="pr_ps", bufs=1, space="PSUM") as pr_ps, \
                 tc.tile_pool(name="prt_ps", bufs=2, space="PSUM") as prt_ps:
                a_t = pra.tile([128, HG], F16)
                b_t = pra.tile([128, HG], F16)

                def load_xg(s):
                    # partition 32j+b <- xgd[32s+b, 1536j:1536(j+1)]
                    xg_sb = prx.tile([128, GC], F16, tag="xgs")
                    for j in range(NG):
                        nc.sync.dma_start(
                            out=xg_sb[BLOC * j:BLOC * (j + 1), :],
                            in_=xgd[BLOC * s:BLOC * (s + 1),
                                    GC * j:GC * (j + 1)])
                    return xg_sb

                # xg for steps 0/1 first so step 0's chain and step 1's
                # identity adds run while Wr streams in behind them
                xg_tiles = {s: load_xg(s) for s in range(min(2, n_steps))}

                # Wr split: [i|g] cols (needed first) and [o] cols, each
                # loaded as two k-halves on both HW DMA queues
                wr01 = prw.tile([128, KT_H, NG, 1024], F16, tag="wr01")
                wr2 = prw.tile([128, KT_H, NG, 512], F16, tag="wr2")
                wr_v = Wr[:, :].rearrange("(k p) (j c) -> p k j c",
                                          p=128, j=NG)
                kh = KT_H // 2
                for j in range(NG):
                    nc.sync.dma_start(out=wr01[:, 0:kh, j, :],
                                      in_=wr_v[:, 0:kh, j, 0:1024])
                    nc.scalar.dma_start(out=wr01[:, kh:KT_H, j, :],
                                        in_=wr_v[:, kh:KT_H, j, 0:1024])
                for j in range(NG):
                    nc.sync.dma_start(out=wr2[:, 0:kh, j, :],
                                      in_=wr_v[:, 0:kh, j, 1024:1536])
                    nc.scalar.dma_start(out=wr2[:, kh:KT_H, j, :],
                                        in_=wr_v[:, kh:KT_H, j, 1024:1536])

                def ident_add(ps, xg_sb, c):
                    """Open bank c's PSUM accumulation with ps = xg (identity
                    matmul); needs only xg, so next step's i/g adds fill the
                    inter-step PE gap."""
                    nc.tensor.matmul(
                        ps[:], lhsT=ident[:],
                        rhs=xg_sb[:, 512 * c:512 * (c + 1)],
                        start=True, stop=False, skip_group_check=True)

                def bank_slots(ps, c, lhs_of, ki_lo, ki_hi):
                    wr_t = wr01 if c < 2 else wr2
                    co = 512 * c if c < 2 else 0
                    for ki in range(ki_lo, ki_hi):
                        k = K_ORDER[ki]
                        lhs = lhs_of(k)
                        for j in range(NG):
                            nc.tensor.matmul(
                                ps[BLOC * j:BLOC * (j + 1), :],
                                lhsT=lhs,
                                rhs=wr_t[:, k, j, co:co + 512],
                                start=False, stop=(ki == KT_H - 1),
                                tile_position=(0, BLOC * j),
                                skip_group_check=True)

                def bank_k_mms(ps, c, hT):
                    bank_slots(ps, c,
                               lambda k: hT[k % NG][:, BLOC * (k // NG):
                                                    BLOC * (k // NG) + BLOC],
                               0, KT_H)

                def act_head(gi_ap, gg_ap):
                    """a_t = tanh(sig(i) * tanh(g)); runs under bank o's
                    matmuls."""
                    nc.scalar.activation(a_t[:], gi_ap,
                                         mybir.ActivationFunctionType.Sigmoid)
                    nc.scalar.activation(b_t[:], gg_ap,
                                         mybir.ActivationFunctionType.Tanh)
                    nc.vector.tensor_mul(a_t[:], a_t[:], b_t[:])
                    nc.scalar.activation(a_t[:], a_t[:],
                                         mybir.ActivationFunctionType.Tanh)

                def act_tail(go_tile, go_off, store_out, interleave=None):
                    """h = sig(o) * a_t, per-chunk, PE-transposed into 4 hT
                    chunks (kept on the PE: low latency + keeps HAM warm); or
                    the final h stored.  interleave(q, hT_q) emits the next
                    step's bank-0 slot group for chunk q right after its
                    transpose, keeping the PE stream continuous."""
                    if store_out:
                        nc.scalar.activation(
                            b_t[:], go_tile[:, go_off:go_off + HG],
                            mybir.ActivationFunctionType.Sigmoid)
                        h_t = pra.tile([128, HG], F32, tag="hfin")
                        nc.vector.tensor_mul(h_t[:], b_t[:], a_t[:])
                        for j in range(NG):
                            nc.sync.dma_start(
                                out=out[:, HID + HG * j:HID + HG * (j + 1)],
                                in_=h_t[BLOC * j:BLOC * (j + 1), :])
                        return None
                    hTs = []
                    for q in range(NG):
                        nc.scalar.activation(
                            b_t[:, 128 * q:128 * (q + 1)],
                            go_tile[:, go_off + 128 * q:go_off + 128 * (q + 1)],
                            mybir.ActivationFunctionType.Sigmoid)
                        h_q = prh.tile([128, 128], F16, tag="h")
                        nc.vector.tensor_mul(h_q[:],
                                             b_t[:, 128 * q:128 * (q + 1)],
                                             a_t[:, 128 * q:128 * (q + 1)])
                        t_ps = prt_ps.tile([128, 128], F16, space="PSUM",
                                           tag="tps")
                        nc.tensor.transpose(out=t_ps[:], in_=h_q[:],
                                            identity=ident[:])
                        hT_q = prt.tile([128, 128], F16, tag="hT")
                        nc.vector.tensor_copy(hT_q[:], t_ps[:])
                        hTs.append(hT_q)
                        # lag the interleaved bank-0 group one chunk behind
                        # the transposes so each group's stationary copy has
                        # a full transpose-time to land (no PSUM-copy bubble)
                        if interleave is not None and q >= 1:
                            interleave(q - 1, hTs[q - 1])
                    if interleave is not None:
                        interleave(NG - 1, hTs[NG - 1])
                    return hTs

                def alloc_ps01():
                    ps = []
                    for c in range(2):
                        gps = pr_ps.tile([128, 512], F32, space="PSUM",
                                         tag=f"gps{c}")
                        ps.append(gps)
                    return ps

                def b0_interleave(ps0):
                    # bank 0 of the next step, one slot group per hT chunk:
                    # group q's slots K_ORDER[4q:4q+4] all consume chunk q
                    def cb(q, hT_q):
                        bank_slots(ps0, 0,
                                   lambda k: hT_q[:, BLOC * (k // NG):
                                                  BLOC * (k // NG) + BLOC],
                                   4 * q, 4 * (q + 1))
                    return cb

                # step 1's i/g identity adds run during the Wr load / step 0
                if n_steps > 1:
                    ps_cur = alloc_ps01()
                    ident_add(ps_cur[0], xg_tiles[1], 0)
                    ident_add(ps_cur[1], xg_tiles[1], 1)

                # step 0: h = 0 -> gates are just xg
                xg0 = xg_tiles[0]
                act_head(xg0[:, 0:HG], xg0[:, HG:2 * HG])
                hT = act_tail(xg0, 2 * HG, store_out=(n_steps == 1),
                              interleave=(b0_interleave(ps_cur[0])
                                          if n_steps > 1 else None))

                for s in range(1, n_steps):
                    # bank 0 of step s was already emitted, interleaved into
                    # step s-1's tail
                    xg_sb = xg_tiles.pop(s)
                    if s + 1 < n_steps:
                        xg_tiles[s + 1] = load_xg(s + 1)
                    ps_b = ps_cur
                    bank_k_mms(ps_b[1], 1, hT)
                    act_head(ps_b[0][:], ps_b[1][:])
                    gps2 = pr_ps.tile([128, 512], F32, space="PSUM",
                                      tag="gps2")
                    ident_add(gps2, xg_sb, 2)
                    bank_k_mms(gps2, 2, hT)
                    if s + 1 < n_steps:
                        # next step's i/g identity adds fill the tail gap
                        ps_cur = alloc_ps01()
                        ident_add(ps_cur[0], xg_tiles[s + 1], 0)
                        ident_add(ps_cur[1], xg_tiles[s + 1], 1)
                        hT = act_tail(gps2, 0, store_out=False,
                                      interleave=b0_interleave(ps_cur[0]))
                    else:
                        hT = act_tail(gps2, 0, store_out=True)
    nc.compile()
    return nc


_BUILT = {}


def _get_built(n_steps=None):
    key = n_steps or N_STEPS
    if key not in _BUILT:
        _BUILT[key] = build(key)
    return _BUILT[key]


def _perm():
    """Row permutation taking PyTorch (i|f|g|o)*2048 rows to 4 groups of
    (i|g|o)*512."""
    p = []
    for j in range(NG):
        for base in (0, 2 * HID, 3 * HID):  # i, g, o blocks
            p.extend(range(base + HG * j, base + HG * (j + 1)))
    return np.array(p)


def prep_inputs(inputs, embed_table, W_ih_f, W_hh_f, b_ih_f, b_hh_f,
                W_ih_b, W_hh_b, b_ih_b, b_hh_b):
    perm = _perm()
    idx = np.asarray(inputs)
    idx = np.where(idx > VOCAB, 0, idx).astype(np.int64)
    idx = np.clip(idx, 0, VOCAB - 1).astype(np.int32)

    Wi_p = np.ascontiguousarray(np.asarray(W_ih_b)[perm].T.astype(np.float16))
    Wf_p = np.ascontiguousarray(np.asarray(W_ih_f)[perm].T.astype(np.float16))
    Wr_p = np.ascontiguousarray(np.asarray(W_hh_b)[perm].T.astype(np.float16))
    bb = (np.asarray(b_ih_b) + np.asarray(b_hh_b))[perm].astype(np.float32)
    bf = (np.asarray(b_ih_f) + np.asarray(b_hh_f))[perm].astype(np.float32)
    bias_b_t = np.ascontiguousarray(np.broadcast_to(bb, (128, G3)))
    # bias_f packed [3, 128, 512]: partition 32j+b -> gate (c, group j)
    bias_f_t = np.empty((3, 128, 512), np.float32)
    for c in range(3):
        for j in range(NG):
            bias_f_t[c, BLOC * j:BLOC * (j + 1), :] = \
                bf[GC * j + 512 * c:GC * j + 512 * (c + 1)]
    table = np.ascontiguousarray(np.asarray(embed_table).astype(np.float16))

    in_maps = []
    for c in range(NCORES):
        sl = idx[BLOC * c:BLOC * (c + 1)]          # [32, 128]
        tok = np.ascontiguousarray(sl[:, ::-1].T.reshape(NTOK, 1))  # t-major rev
        in_maps.append({
            "tok": tok, "table": table, "Wi": Wi_p, "Wf": Wf_p, "Wr": Wr_p,
            "bias_b": bias_b_t, "bias_f": bias_f_t,
        })
    return in_maps


def kernel(**inputs) -> np.ndarray:
    from concourse.bass_utils import run_bass_kernel_spmd
    nc = _get_built()
    in_maps = prep_inputs(**inputs)
    res = run_bass_kernel_spmd(nc, in_maps, core_ids=list(range(NCORES)))
    return np.concatenate([res.results[c]["out"] for c in range(NCORES)], axis=0)
